# revision 1
# baseline (speedup 1.0000x reference)
"""Local window attention (7x7 windows, 8 heads, d=64) Trainium2 Bass kernel.

Full inputs in, full outputs out. Internally: data-parallel over batch across
8 NeuronCores (4 images per core). All shapes hardcoded per the problem spec:
  fmap (32, 56, 56, 256) f32, Wq (256,512), Wkv (256,1024), Wo (512,256), bo (256,)

Per-core dataflow (one "group" = 2 adjacent-y windows = 98 tokens, padded to
2x64 token slots on partitions so window w sits at partitions 64w..64w+48):
  f_raw [128,256]  <- DMA (2 windows)
  fT    [128,2,128](PE transpose)  c-on-partition
  qT,kT [128,4,98] = W.T @ fT      (4 n-chunks of 128, tokens compact 2x49)
  v     [128,512]  = f @ Wv        (token-padded rows)
  ST    [128,4,2,49] psum: per (chunk,hp,w): kT.T @ qT -> S^T [j,i]
  expS  = exp(SCALE * ST)          one ACT op
  out'  [128,2,2,65] psum x2: expS.T @ [v | ones] -> [i, 64+denom]
  out   [128,512] = out' * recip(denom)  (token-padded rows, head-major cols)
  outT  (PE transpose x4) -> final = outT.T @ Wo + bo -> DMA out
"""

from contextlib import ExitStack

import numpy as np

import concourse.bacc as bacc
import concourse.bass as bass
import concourse.tile as tile
from concourse import mybir
from concourse.masks import make_identity
from concourse.bass_utils import run_bass_kernel_spmd

P = 7
PP = 49          # tokens per window
H = 8            # heads
D = 64           # head dim
DIM = 256        # channels
INNER = 512      # h*d
SCALE = D ** -0.5
IMGS_PER_CORE = 4
NCORES = 8
X = 56
NW = X // P      # 8 windows per axis
FP32 = mybir.dt.float32


def build_bass(n_imgs=IMGS_PER_CORE):
    nc = bacc.Bacc("TRN2", target_bir_lowering=False, debug=False)

    fm = nc.dram_tensor("fmap", [n_imgs, X, X, DIM], FP32, kind="ExternalInput").ap()
    wq = nc.dram_tensor("Wq", [DIM, INNER], FP32, kind="ExternalInput").ap()
    wkv = nc.dram_tensor("Wkv", [DIM, 2 * INNER], FP32, kind="ExternalInput").ap()
    wo = nc.dram_tensor("Wo", [INNER, DIM], FP32, kind="ExternalInput").ap()
    bo = nc.dram_tensor("bo", [DIM], FP32, kind="ExternalInput").ap()
    out = nc.dram_tensor("out", [n_imgs, X, X, DIM], FP32, kind="ExternalOutput").ap()

    with tile.TileContext(nc) as tc:
        with ExitStack() as ctx:
            build_kernel(ctx, tc, out, fm, wq, wkv, wo, bo, n_imgs)
    nc.compile()
    return nc


def build_kernel(ctx, tc, out, fm, wq, wkv, wo, bo, n_imgs=IMGS_PER_CORE):
    nc = tc.nc
    consts = ctx.enter_context(tc.tile_pool(name="consts", bufs=1))
    sb = ctx.enter_context(tc.tile_pool(name="sb", bufs=3))
    ps = ctx.enter_context(tc.tile_pool(name="ps", bufs=8, space="PSUM"))

    # ---- constants ----
    ident = consts.tile([128, 128], FP32)
    make_identity(nc, ident[:])

    ones = consts.tile([128, 128], FP32)
    nc.gpsimd.memset(ones[:], 1.0)

    # weights, contraction dim (input channels) on partitions, chunked by 128
    wq_s = consts.tile([128, 2, INNER], FP32)   # [ck, kc, n]
    nc.sync.dma_start(out=wq_s[:], in_=wq.rearrange("(kc ck) n -> ck kc n", ck=128))
    wk_s = consts.tile([128, 2, INNER], FP32)
    nc.sync.dma_start(out=wk_s[:], in_=wkv[:, 0:INNER].rearrange("(kc ck) n -> ck kc n", ck=128))
    wv_s = consts.tile([128, 2, INNER], FP32)
    nc.sync.dma_start(out=wv_s[:], in_=wkv[:, INNER:2 * INNER].rearrange("(kc ck) n -> ck kc n", ck=128))
    wo_s = consts.tile([128, 4, DIM], FP32)     # [ck, kc, m]
    nc.sync.dma_start(out=wo_s[:], in_=wo.rearrange("(kc ck) m -> ck kc m", ck=128))
    bo_s = consts.tile([1, DIM], FP32)
    nc.sync.dma_start(out=bo_s[:], in_=bo[None, :])

    # ---- main loop: one group = 2 windows (same wx, adjacent wy) ----
    for img in range(n_imgs):
        for wx in range(NW):
            for u in range(NW // 2):
                group(nc, sb, ps, out, fm, wq_s, wk_s, wv_s, wo_s, bo_s, ident, ones,
                      img, wx, u)


def group(nc, sb, ps, out, fm, wq_s, wk_s, wv_s, wo_s, bo_s, ident, ones, img, wx, u):
    # 1. load 2 windows, token-padded: window w tokens at partitions 64w..64w+48
    f_raw = sb.tile([128, DIM], FP32, tag="f_raw")
    for w in range(2):
        wy = 2 * u + w
        for r in range(P):
            nc.sync.dma_start(
                out=f_raw[64 * w + P * r:64 * w + P * r + P, :],
                in_=fm[img, P * wx + r, P * wy:P * wy + P, :],
            )

    def ps_tile(shape):
        # uniform bank-sized psum slots; view-slice to the requested shape
        t = ps.tile([128, 512], FP32, tag="ps")
        n = int(np.prod(shape[1:]))
        v_ = t[:, 0:n]
        if len(shape) > 2:
            dims = " ".join(f"d{i}" for i in range(1, len(shape)))
            v_ = v_.rearrange(f"p ({dims}) -> p {dims}",
                              **{f"d{i}": shape[i] for i in range(1, len(shape) - 1)})
        return v_

    # 2-3. transpose -> fT [ck, kc, t]  (c on partitions, tokens padded on free)
    fT_ps = ps_tile([128, 2, 128])
    for kc in range(2):
        nc.tensor.transpose(fT_ps[:, kc, :], f_raw[:, 128 * kc:128 * kc + 128], ident[:])
    fT = sb.tile([128, 2, 128], FP32, tag="fT")
    nc.scalar.copy(fT[:], fT_ps[:])

    # 4-5. qT, kT [nc*128, 2x64 padded] = W.T @ fT
    qT_ps = ps_tile([128, 4, 128])
    kT_ps = ps_tile([128, 4, 128])
    for nk in range(4):
        for kc in range(2):
            nc.tensor.matmul(qT_ps[:, nk, :], wq_s[:, kc, 128 * nk:128 * nk + 128],
                             fT[:, kc, :], start=(kc == 0), stop=(kc == 1))
            nc.tensor.matmul(kT_ps[:, nk, :], wk_s[:, kc, 128 * nk:128 * nk + 128],
                             fT[:, kc, :], start=(kc == 0), stop=(kc == 1))
    # HW bug: matmul operands must start at partition 0 (high-half streaming
    # is broken), so split head-parities into base-0 tiles during the
    # mandatory psum->SBUF copies.
    qT = sb.tile([64, 4, 2, 128], FP32, tag="qT")   # [d, ch, hp, t]
    kT = sb.tile([64, 4, 2, 128], FP32, tag="kT")
    for hp in range(2):
        nc.vector.tensor_copy(qT[:, :, hp, :], qT_ps[64 * hp:64 * hp + 64, :, :])
        nc.scalar.copy(kT[:, :, hp, :], kT_ps[64 * hp:64 * hp + 64, :, :])

    # 6-7. v [t(padded), 512] = f @ Wv
    v_ps = ps_tile([128, INNER])
    for kc in range(2):
        nc.tensor.matmul(v_ps[:], fT[:, kc, :], wv_s[:, kc, :],
                         start=(kc == 0), stop=(kc == 1))
    v = sb.tile([64, 2, INNER], FP32, tag="v")      # [j, w, n]
    for w in range(2):
        nc.vector.tensor_copy(v[:, w, :], v_ps[64 * w:64 * w + 64, :])

    # 8-9. S^T then exp:  ST[j@64w, (ch, hp, i)]  (i padded to 64)
    # lhsT = kT slice with M=64 (incl. 15 pad cols) so psum rows are fully
    # written; pad lanes carry junk that is never consumed.
    st_ps = ps_tile([128, 4, 2, 64])
    for ch in range(4):
        for hp in range(2):
            for w in range(2):
                nc.tensor.matmul(
                    st_ps[64 * w:64 * w + 64, ch, hp, :],
                    kT[:, ch, hp, 64 * w:64 * w + 64],
                    qT[:, ch, hp, 64 * w:64 * w + 64],
                    tile_position=(0, 64 * w),
                )
    expS = sb.tile([64, 2, 4, 2, 64], FP32, tag="expS")  # [j, w, ch, hp, i]
    for w in range(2):
        nc.scalar.activation(expS[:, w, :, :, :], st_ps[64 * w:64 * w + 64, :, :, :],
                             mybir.ActivationFunctionType.Exp, scale=SCALE)

    # 10. out' = expS.T @ [v | 1]:   av[i@64w, (chL, hp, d|denom)]
    av_tiles = []
    for chpair in range(2):
        av = ps_tile([128, 2, 2, D + 1])
        av_tiles.append(av)
        for chL in range(2):
            ch = 2 * chpair + chL
            for hp in range(2):
                h = 2 * ch + hp
                for w in range(2):
                    # lhsT: K = 49 real keys (base 0), M = 64 (incl. pad
                    # queries so psum rows are fully written)
                    e = expS[0:PP, w, ch, hp, :]
                    nc.tensor.matmul(av[64 * w:64 * w + 64, chL, hp, 0:D],
                                     e, v[0:PP, w, D * h:D * h + D],
                                     tile_position=(0, 64 * w))
                    nc.tensor.matmul(av[64 * w:64 * w + 64, chL, hp, D:D + 1],
                                     e, ones[0:PP, 0:1],
                                     tile_position=(0, 64 * w))

    # 11-12. normalize: out_tok [t(padded), h*64+d]
    out_tok = sb.tile([128, INNER], FP32, tag="out_tok")
    for chpair in range(2):
        av = av_tiles[chpair]
        recd = sb.tile([128, 2, 2], FP32, tag="recd")
        nc.vector.reciprocal(recd[:], av[:, :, :, D])
        for chL in range(2):
            for hp in range(2):
                h = 2 * (2 * chpair + chL) + hp
                nc.vector.tensor_scalar(
                    out=out_tok[:, D * h:D * h + D],
                    in0=av[:, chL, hp, 0:D],
                    scalar1=recd[:, chL, hp:hp + 1],
                    scalar2=None,
                    op0=mybir.AluOpType.mult,
                )

    # 13-14. transpose out_tok -> outT [n, t(padded)]
    ot_ps = ps_tile([128, 4, 128])
    for nk in range(4):
        nc.tensor.transpose(ot_ps[:, nk, :], out_tok[:, 128 * nk:128 * nk + 128],
                            ident[:])
    outT = sb.tile([128, 4, 128], FP32, tag="outT")
    nc.scalar.copy(outT[:], ot_ps[:])

    # 15. final = outT.T @ Wo + bo   [t(padded), 256]
    fin_ps = ps_tile([128, DIM])
    for nk in range(4):
        nc.tensor.matmul(fin_ps[:], outT[:, nk, :], wo_s[:, nk, :],
                         start=(nk == 0), stop=False)
    nc.tensor.matmul(fin_ps[:], ones[0:1, 0:128], bo_s[:], start=False, stop=True)
    fin = sb.tile([128, DIM], FP32, tag="fin")
    nc.vector.tensor_copy(fin[:], fin_ps[:])

    # 16. store
    for w in range(2):
        wy = 2 * u + w
        for r in range(P):
            nc.sync.dma_start(
                out=out[img, P * wx + r, P * wy:P * wy + P, :],
                in_=fin[64 * w + P * r:64 * w + P * r + P, :],
            )


_CACHED = {}


def _get_nc():
    if "nc" not in _CACHED:
        _CACHED["nc"] = build_bass()
    return _CACHED["nc"]


def kernel(fmap, Wq, Wkv, Wo, bo, _trace=False, _trace_kwargs=None):
    fmap = np.ascontiguousarray(fmap, dtype=np.float32)
    nc = _get_nc()
    in_maps = []
    for c in range(NCORES):
        in_maps.append({
            "fmap": fmap[IMGS_PER_CORE * c:IMGS_PER_CORE * (c + 1)],
            "Wq": np.ascontiguousarray(Wq, dtype=np.float32),
            "Wkv": np.ascontiguousarray(Wkv, dtype=np.float32),
            "Wo": np.ascontiguousarray(Wo, dtype=np.float32),
            "bo": np.ascontiguousarray(bo, dtype=np.float32),
        })
    res = run_bass_kernel_spmd(nc, in_maps, core_ids=list(range(NCORES)),
                               trace=_trace, **(_trace_kwargs or {}))
    outs = [r["out"] for r in res.results]
    full = np.concatenate(outs, axis=0)
    if _trace:
        return full, res
    return full



# revision 4
# speedup vs baseline: 1.0091x; 1.0091x over previous
"""Local window attention (7x7 windows, 8 heads, d=64) Trainium2 Bass kernel.

Full inputs in, full outputs out. Internally: data-parallel over batch across
8 NeuronCores (4 images per core). All shapes hardcoded per the problem spec:
  fmap (32, 56, 56, 256) f32, Wq (256,512), Wkv (256,1024), Wo (512,256), bo (256,)

Per-core dataflow (one "group" = 2 adjacent-y windows = 98 tokens, padded to
2x64 token slots on partitions so window w sits at partitions 64w..64w+48).
All matmuls run in bf16 (1 cycle/row on PE vs fp32's 4); psum accumulation
stays fp32. Casts are folded into the mandatory psum->SBUF copies.
  f_raw [128,256]  <- DMA (2 windows)
  fT    [128,2,128](PE transpose, fp32)  c-on-partition, cast to bf16 on copy
  qT,kT [128,4,98] = W.T @ fT      (4 n-chunks of 128, tokens compact 2x49)
  v     [128,512]  = f @ Wv        (token-padded rows)
  ST    [128,4,2,49] psum: per (chunk,hp,w): kT.T @ qT -> S^T [j,i]
  expS  = exp(SCALE * ST)          one ACT op, out bf16
  out'  [128,2,2,65] psum x2: expS.T @ [v | ones] -> [i, 64+denom]
  out   [128,512] = out' * recip(denom)  (token-padded rows, head-major cols)
  outT  (PE transpose x4, fp32) -> final = outT.T @ Wo (+bo via DVE) -> DMA out
"""

from contextlib import ExitStack

import numpy as np

import concourse.bacc as bacc
import concourse.bass as bass
import concourse.tile as tile
from concourse import mybir
from concourse.masks import make_identity
from concourse.bass_utils import run_bass_kernel_spmd

P = 7
PP = 49          # tokens per window
H = 8            # heads
D = 64           # head dim
DIM = 256        # channels
INNER = 512      # h*d
SCALE = D ** -0.5
IMGS_PER_CORE = 4
NCORES = 8
X = 56
NW = X // P      # 8 windows per axis
FP32 = mybir.dt.float32
BF16 = mybir.dt.bfloat16


def build_bass(n_imgs=IMGS_PER_CORE):
    nc = bacc.Bacc("TRN2", target_bir_lowering=False, debug=False)

    fm = nc.dram_tensor("fmap", [n_imgs, X, X, DIM], FP32, kind="ExternalInput").ap()
    wq = nc.dram_tensor("Wq", [DIM, INNER], FP32, kind="ExternalInput").ap()
    wkv = nc.dram_tensor("Wkv", [DIM, 2 * INNER], FP32, kind="ExternalInput").ap()
    wo = nc.dram_tensor("Wo", [INNER, DIM], FP32, kind="ExternalInput").ap()
    bo = nc.dram_tensor("bo", [DIM], FP32, kind="ExternalInput").ap()
    out = nc.dram_tensor("out", [n_imgs, X, X, DIM], FP32, kind="ExternalOutput").ap()

    with tile.TileContext(nc) as tc:
        with ExitStack() as ctx:
            build_kernel(ctx, tc, out, fm, wq, wkv, wo, bo, n_imgs)
    nc.compile()
    return nc


def build_kernel(ctx, tc, out, fm, wq, wkv, wo, bo, n_imgs=IMGS_PER_CORE):
    nc = tc.nc
    consts = ctx.enter_context(tc.tile_pool(name="consts", bufs=1))
    sb = ctx.enter_context(tc.tile_pool(name="sb", bufs=3))
    ps = ctx.enter_context(tc.tile_pool(name="ps", bufs=8, space="PSUM"))

    # ---- constants ----
    ident = consts.tile([128, 128], FP32)
    make_identity(nc, ident[:])

    ones = consts.tile([128, 128], FP32)
    nc.gpsimd.memset(ones[:], 1.0)
    ones_b = consts.tile([128, 1], BF16)
    nc.gpsimd.memset(ones_b[:], 1.0)

    # weights staged fp32, cast once to bf16 working copies
    # (contraction dim on partitions, chunked by 128)
    wq_f = consts.tile([128, 2, INNER], FP32)   # [ck, kc, n]
    nc.sync.dma_start(out=wq_f[:], in_=wq.rearrange("(kc ck) n -> ck kc n", ck=128))
    wk_f = consts.tile([128, 2, INNER], FP32)
    nc.sync.dma_start(out=wk_f[:], in_=wkv[:, 0:INNER].rearrange("(kc ck) n -> ck kc n", ck=128))
    wv_f = consts.tile([128, 2, INNER], FP32)
    nc.sync.dma_start(out=wv_f[:], in_=wkv[:, INNER:2 * INNER].rearrange("(kc ck) n -> ck kc n", ck=128))
    wo_f = consts.tile([128, 4, DIM], FP32)     # [ck, kc, m]
    nc.sync.dma_start(out=wo_f[:], in_=wo.rearrange("(kc ck) m -> ck kc m", ck=128))
    bo_f = consts.tile([1, DIM], FP32)
    nc.sync.dma_start(out=bo_f[:], in_=bo[None, :])

    wq_s = consts.tile([128, 2, INNER], BF16)
    nc.vector.tensor_copy(wq_s[:], wq_f[:])
    wk_s = consts.tile([128, 2, INNER], BF16)
    nc.vector.tensor_copy(wk_s[:], wk_f[:])
    wv_s = consts.tile([128, 2, INNER], BF16)
    nc.vector.tensor_copy(wv_s[:], wv_f[:])
    wo_s = consts.tile([128, 4, DIM], BF16)
    nc.vector.tensor_copy(wo_s[:], wo_f[:])

    # bias broadcast to all 128 partitions once (PE outer product ones x bo)
    bb_ps = ps.tile([128, 512], FP32, tag="ps")
    nc.tensor.matmul(bb_ps[:, 0:DIM], ones[0:1, :], bo_f[:], start=True, stop=True)
    bo_bc = consts.tile([128, DIM], FP32)
    nc.scalar.copy(bo_bc[:], bb_ps[:, 0:DIM])

    # ---- main loop: one group = 2 windows (same wx, adjacent wy) ----
    for img in range(n_imgs):
        for wx in range(NW):
            for u in range(NW // 2):
                group(nc, sb, ps, out, fm, wq_s, wk_s, wv_s, wo_s, bo_bc, ident,
                      ones_b, img, wx, u)


def group(nc, sb, ps, out, fm, wq_s, wk_s, wv_s, wo_s, bo_bc, ident, ones_b,
          img, wx, u):
    # 1. load 2 windows, token-padded: window w tokens at partitions 64w..64w+48
    f_raw = sb.tile([128, DIM], FP32, tag="f_raw")
    for w in range(2):
        wy = 2 * u + w
        for r in range(P):
            nc.sync.dma_start(
                out=f_raw[64 * w + P * r:64 * w + P * r + P, :],
                in_=fm[img, P * wx + r, P * wy:P * wy + P, :],
            )

    def ps_tile(shape):
        # uniform bank-sized psum slots; view-slice to the requested shape
        t = ps.tile([128, 512], FP32, tag="ps")
        n = int(np.prod(shape[1:]))
        v_ = t[:, 0:n]
        if len(shape) > 2:
            dims = " ".join(f"d{i}" for i in range(1, len(shape)))
            v_ = v_.rearrange(f"p ({dims}) -> p {dims}",
                              **{f"d{i}": shape[i] for i in range(1, len(shape) - 1)})
        return v_

    # 2-3. transpose -> fT [ck, kc, t]  (c on partitions, tokens padded on free)
    fT_ps = ps_tile([128, 2, 128])
    for kc in range(2):
        nc.tensor.transpose(fT_ps[:, kc, :], f_raw[:, 128 * kc:128 * kc + 128], ident[:])
    fT = sb.tile([128, 2, 128], BF16, tag="fT")
    nc.scalar.copy(fT[:], fT_ps[:])

    # 4-5. qT, kT [nc*128, 2x64 padded] = W.T @ fT
    qT_ps = ps_tile([128, 4, 128])
    kT_ps = ps_tile([128, 4, 128])
    for nk in range(4):
        for kc in range(2):
            nc.tensor.matmul(qT_ps[:, nk, :], wq_s[:, kc, 128 * nk:128 * nk + 128],
                             fT[:, kc, :], start=(kc == 0), stop=(kc == 1))
            nc.tensor.matmul(kT_ps[:, nk, :], wk_s[:, kc, 128 * nk:128 * nk + 128],
                             fT[:, kc, :], start=(kc == 0), stop=(kc == 1))
    # HW bug: matmul operands must start at partition 0 (high-half streaming
    # is broken), so split head-parities into base-0 tiles during the
    # mandatory psum->SBUF copies.
    qT = sb.tile([64, 4, 2, 128], BF16, tag="qT")   # [d, ch, hp, t]
    kT = sb.tile([64, 4, 2, 128], BF16, tag="kT")
    for hp in range(2):
        nc.vector.tensor_copy(qT[:, :, hp, :], qT_ps[64 * hp:64 * hp + 64, :, :])
        nc.scalar.copy(kT[:, :, hp, :], kT_ps[64 * hp:64 * hp + 64, :, :])

    # 6-7. v [t(padded), 512] = f @ Wv
    v_ps = ps_tile([128, INNER])
    for kc in range(2):
        nc.tensor.matmul(v_ps[:], fT[:, kc, :], wv_s[:, kc, :],
                         start=(kc == 0), stop=(kc == 1))
    v = sb.tile([64, 2, INNER], BF16, tag="v")      # [j, w, n]
    nc.vector.tensor_copy(v[:, 0, :], v_ps[0:64, :])
    nc.scalar.copy(v[:, 1, :], v_ps[64:128, :])

    # 8-9. S^T then exp:  ST[j@64w, (ch, hp, i)]  (i padded to 64)
    # lhsT = kT slice with M=64 (incl. 15 pad cols) so psum rows are fully
    # written; pad lanes carry junk that is never consumed.
    st_ps = ps_tile([128, 4, 2, 64])
    for ch in range(4):
        for hp in range(2):
            for w in range(2):
                nc.tensor.matmul(
                    st_ps[64 * w:64 * w + 64, ch, hp, :],
                    kT[:, ch, hp, 64 * w:64 * w + 64],
                    qT[:, ch, hp, 64 * w:64 * w + 64],
                    tile_position=(0, 64 * w),
                )
    expS = sb.tile([64, 2, 4, 2, 64], BF16, tag="expS")  # [j, w, ch, hp, i]
    for w in range(2):
        nc.scalar.activation(expS[:, w, :, :, :], st_ps[64 * w:64 * w + 64, :, :, :],
                             mybir.ActivationFunctionType.Exp, scale=SCALE)

    # 10. out' = expS.T @ [v | 1]:   av[i@64w, (chL, hp, d|denom)]
    av_tiles = []
    for chpair in range(2):
        av = ps_tile([128, 2, 2, D + 1])
        av_tiles.append(av)
        for chL in range(2):
            ch = 2 * chpair + chL
            for hp in range(2):
                h = 2 * ch + hp
                for w in range(2):
                    # lhsT: K = 49 real keys (base 0), M = 64 (incl. pad
                    # queries so psum rows are fully written)
                    e = expS[0:PP, w, ch, hp, :]
                    nc.tensor.matmul(av[64 * w:64 * w + 64, chL, hp, 0:D],
                                     e, v[0:PP, w, D * h:D * h + D],
                                     tile_position=(0, 64 * w))
                    nc.tensor.matmul(av[64 * w:64 * w + 64, chL, hp, D:D + 1],
                                     e, ones_b[0:PP, 0:1],
                                     tile_position=(0, 64 * w))

    # 11-12. normalize: out_tok [t(padded), h*64+d]  (fp32, transposed next)
    out_tok = sb.tile([128, INNER], FP32, tag="out_tok")
    for chpair in range(2):
        av = av_tiles[chpair]
        recd = sb.tile([128, 2, 2], FP32, tag="recd")
        nc.vector.reciprocal(recd[:], av[:, :, :, D])
        for chL in range(2):
            for hp in range(2):
                h = 2 * (2 * chpair + chL) + hp
                nc.vector.tensor_scalar(
                    out=out_tok[:, D * h:D * h + D],
                    in0=av[:, chL, hp, 0:D],
                    scalar1=recd[:, chL, hp:hp + 1],
                    scalar2=None,
                    op0=mybir.AluOpType.mult,
                )

    # 13-14. transpose out_tok -> outT [n, t(padded)], cast bf16 on copy
    ot_ps = ps_tile([128, 4, 128])
    for nk in range(4):
        nc.tensor.transpose(ot_ps[:, nk, :], out_tok[:, 128 * nk:128 * nk + 128],
                            ident[:])
    outT = sb.tile([128, 4, 128], BF16, tag="outT")
    nc.scalar.copy(outT[:], ot_ps[:])

    # 15. final = outT.T @ Wo  [t(padded), 256];  + bo fused into psum copy
    fin_ps = ps_tile([128, DIM])
    for nk in range(4):
        nc.tensor.matmul(fin_ps[:], outT[:, nk, :], wo_s[:, nk, :],
                         start=(nk == 0), stop=(nk == 3))
    fin = sb.tile([128, DIM], FP32, tag="fin")
    nc.vector.scalar_tensor_tensor(out=fin[:], in0=fin_ps[:], scalar=1.0,
                                   in1=bo_bc[:], op0=mybir.AluOpType.mult,
                                   op1=mybir.AluOpType.add)

    # 16. store
    for w in range(2):
        wy = 2 * u + w
        for r in range(P):
            nc.sync.dma_start(
                out=out[img, P * wx + r, P * wy:P * wy + P, :],
                in_=fin[64 * w + P * r:64 * w + P * r + P, :],
            )


_CACHED = {}


def _get_nc():
    if "nc" not in _CACHED:
        _CACHED["nc"] = build_bass()
    return _CACHED["nc"]


def kernel(fmap, Wq, Wkv, Wo, bo, _trace=False, _trace_kwargs=None):
    fmap = np.ascontiguousarray(fmap, dtype=np.float32)
    nc = _get_nc()
    in_maps = []
    for c in range(NCORES):
        in_maps.append({
            "fmap": fmap[IMGS_PER_CORE * c:IMGS_PER_CORE * (c + 1)],
            "Wq": np.ascontiguousarray(Wq, dtype=np.float32),
            "Wkv": np.ascontiguousarray(Wkv, dtype=np.float32),
            "Wo": np.ascontiguousarray(Wo, dtype=np.float32),
            "bo": np.ascontiguousarray(bo, dtype=np.float32),
        })
    res = run_bass_kernel_spmd(nc, in_maps, core_ids=list(range(NCORES)),
                               trace=_trace, **(_trace_kwargs or {}))
    outs = [r["out"] for r in res.results]
    full = np.concatenate(outs, axis=0)
    if _trace:
        return full, res
    return full


# revision 12
# speedup vs baseline: 1.0279x; 1.0187x over previous
"""Local window attention (7x7 windows, 8 heads, d=64) Trainium2 Bass kernel.

Full inputs in, full outputs out. Data-parallel over batch across 8 cores
(4 images per core). Shapes hardcoded per spec:
  fmap (32, 56, 56, 256) f32, Wq (256,512), Wkv (256,1024), Wo (512,256), bo (256,)

v2 design, per group = 2 adjacent-y windows = 98 COMPACT tokens (p = 49w+7r+t):
  - 2 DMAs in (one per window, [49,256] <- [7,7,256]), 2 DMAs out; DMA
    triggers spread across Sync and GpSimd queues (v1 was sync-bound: 28
    per-row DMAs x 742ns serialized on the sync sequencer).
  - All matmuls bf16 (1 cy/row); psum fp32.
  - fT via 2 PE transposes [98,128] (fp32, 2cy/row); cast on psum->sbuf copy.
  - q/k: psum [128(2 heads hp-split), 4nk, 98]; copies build:
      qT2 [128 d-stack, 4ch, 2w, 49]  (hp h-even on partitions 0:64, h-odd 64:128)
      kT2 [128 d-stack, 2w, 4ch, 128 j-slots] BLOCK-DIAG: h-even keys at
          cols 0:49 (rows 64:128 zero), h-odd keys at cols 64:113 (rows 0:64
          zero). Zero quadrants persist across groups (3 manually-rotated
          pre-zeroed buffers), so ST does 2 heads per matmul:
  - ST: 8 matmuls [128,128]x[128,49] -> st [128 j-hp-stack, (w,ch), 49].
  - exp: 2 ACT ops (rows 0:49 / 64:113) -> expS_hp [49, (w,ch), 49] bf16.
  - softmax denominators on the (otherwise idle) GPSIMD engine:
    partition_all_reduce(add) then in-place divide -> expS normalized.
  - av: 16 matmuls lhsT=expS[49j,49i], rhs=v2[49j,64d] -> av [49i,64] packed
    in ONE psum bank [128(2 hp col-blocks), 2w, 4ch, 64].
  - out_tok [98,512] fp32 -> 4 fp32 transposes -> outT bf16 -> fin = outT.T@Wo
    (+bo fused via scalar_tensor_tensor) -> 2 DMAs out.
"""

from contextlib import ExitStack

import numpy as np

import concourse.bacc as bacc
import concourse.bass as bass
import concourse.tile as tile
from concourse import mybir
from concourse import bass_isa
from concourse.masks import make_identity
from concourse.bass_utils import run_bass_kernel_spmd

P = 7
PP = 49          # tokens per window
H = 8            # heads
D = 64           # head dim
DIM = 256        # channels
INNER = 512      # h*d
SCALE = D ** -0.5
IMGS_PER_CORE = 4
NCORES = 8
X = 56
NW = X // P      # 8 windows per axis
FP32 = mybir.dt.float32
BF16 = mybir.dt.bfloat16
NROT = 3         # manual rotation depth for persistent-zero tiles


def build_bass(n_imgs=IMGS_PER_CORE):
    nc = bacc.Bacc("TRN2", target_bir_lowering=False, debug=False)

    fm = nc.dram_tensor("fmap", [n_imgs, X, X, DIM], FP32, kind="ExternalInput").ap()
    wq = nc.dram_tensor("Wq", [DIM, INNER], FP32, kind="ExternalInput").ap()
    wkv = nc.dram_tensor("Wkv", [DIM, 2 * INNER], FP32, kind="ExternalInput").ap()
    wo = nc.dram_tensor("Wo", [INNER, DIM], FP32, kind="ExternalInput").ap()
    bo = nc.dram_tensor("bo", [DIM], FP32, kind="ExternalInput").ap()
    out = nc.dram_tensor("out", [n_imgs, X, X, DIM], FP32, kind="ExternalOutput").ap()

    with tile.TileContext(nc) as tc:
        with ExitStack() as ctx:
            build_kernel(ctx, tc, out, fm, wq, wkv, wo, bo, n_imgs)
    nc.compile()
    return nc


def build_kernel(ctx, tc, out, fm, wq, wkv, wo, bo, n_imgs=IMGS_PER_CORE):
    nc = tc.nc
    consts = ctx.enter_context(tc.tile_pool(name="consts", bufs=1))
    sb = ctx.enter_context(tc.tile_pool(name="sb", bufs=4))
    ps = ctx.enter_context(tc.tile_pool(name="ps", bufs=8, space="PSUM"))

    # ---- constants ----
    ident = consts.tile([128, 128], FP32)
    make_identity(nc, ident[:])

    ones = consts.tile([1, 128], FP32)
    nc.gpsimd.memset(ones[:], 1.0)

    # weights staged fp32 (rotating sb tiles), cast once to bf16
    def stage(dram_ap, shape, name):
        st = sb.tile(shape, FP32, tag="stage")
        nc.sync.dma_start(out=st[:], in_=dram_ap)
        bt = consts.tile(shape, BF16, tag=name)
        nc.vector.tensor_copy(bt[:], st[:])
        return bt

    wq_s = stage(wq.rearrange("(kc ck) n -> ck kc n", ck=128), [128, 2, INNER],
                 "wq_s")
    wk_s = stage(wkv[:, 0:INNER].rearrange("(kc ck) n -> ck kc n", ck=128),
                 [128, 2, INNER], "wk_s")
    wv_s = stage(wkv[:, INNER:2 * INNER].rearrange("(kc ck) n -> ck kc n", ck=128),
                 [128, 2, INNER], "wv_s")
    wo_s = stage(wo.rearrange("(kc ck) m -> ck kc m", ck=128), [128, 4, DIM],
                 "wo_s")

    bo_f = consts.tile([1, DIM], FP32)
    nc.sync.dma_start(out=bo_f[:], in_=bo[None, :])
    # bias broadcast to 98 partitions once (PE outer product ones x bo)
    bb_ps = ps.tile([128, 512], FP32, tag="ps")
    nc.tensor.matmul(bb_ps[:, 0:DIM], ones[0:1, :], bo_f[:],
                     start=True, stop=True)
    bo_bc = consts.tile([128, DIM], FP32)
    nc.scalar.copy(bo_bc[:], bb_ps[:, 0:DIM])

    # persistent block-diagonal kT2 buffers: zero quadrants must survive
    # rotation, so allocate NROT fixed buffers and memset once.
    kT2_bufs = []
    for i in range(NROT):
        t = consts.tile([128, 2, 4, 128], BF16, tag=f"kT2_{i}")
        nc.gpsimd.memset(t[:], 0.0)
        kT2_bufs.append(t)

    # ---- main loop ----
    gi = 0
    for img in range(n_imgs):
        for wx in range(NW):
            for u in range(NW // 2):
                group(nc, sb, ps, out, fm, wq_s, wk_s, wv_s, wo_s, bo_bc, ident,
                      kT2_bufs[gi % NROT], img, wx, u)
                gi += 1


def group(nc, sb, ps, out, fm, wq_s, wk_s, wv_s, wo_s, bo_bc, ident, kT2,
          img, wx, u):
    r0 = P * wx

    # 1. load 2 windows, 64-padded slots: token p = 64w + 7r + t
    #    (engine partition starts must be 32-aligned, so window base is 64)
    f_raw = sb.tile([128, DIM], FP32, tag="f_raw")
    for w in range(2):
        c0 = P * (2 * u + w)
        nc.sync.dma_start(out=f_raw[64 * w:64 * w + PP, :],
                          in_=fm[img, r0:r0 + P, c0:c0 + P, :])

    # 2. transpose -> fT [c(2x128), 128 tok-slots]; cast bf16 on copy
    fT_bank = ps.tile([128, 512], FP32, tag="ps")
    fT_ps = fT_bank[:, 0:256].rearrange("p (kc t) -> p kc t", kc=2)
    for kc in range(2):
        nc.tensor.transpose(fT_ps[:, kc, :], f_raw[:, 128 * kc:128 * kc + 128],
                            ident[:])
    fT = sb.tile([128, 2, 128], BF16, tag="fT")
    nc.scalar.copy(fT[:], fT_ps[:])

    # 3. q, k projections -> psum [128(hp-split), 4nk, 128 tok-slots]
    q_bank = ps.tile([128, 512], FP32, tag="ps")
    qT_ps = q_bank[:].rearrange("p (nk t) -> p nk t", nk=4)
    k_bank = ps.tile([128, 512], FP32, tag="ps")
    kT_ps = k_bank[:].rearrange("p (nk t) -> p nk t", nk=4)
    for nk in range(4):
        for kc in range(2):
            nc.tensor.matmul(qT_ps[:, nk, :], wq_s[:, kc, 128 * nk:128 * nk + 128],
                             fT[:, kc, :], start=(kc == 0), stop=(kc == 1))
            nc.tensor.matmul(kT_ps[:, nk, :], wk_s[:, kc, 128 * nk:128 * nk + 128],
                             fT[:, kc, :], start=(kc == 0), stop=(kc == 1))

    # qT2 [128 d-stack, 4ch, 2w, 49]: h-even d on partitions 0:64, h-odd 64:128
    qT2 = sb.tile([128, 4, 2, PP], BF16, tag="qT2")
    for hp in range(2):
        nc.vector.tensor_copy(
            qT2[64 * hp:64 * hp + 64, :, :, :],
            qT_ps[64 * hp:64 * hp + 64, :, :].rearrange(
                "p nk (w ts) -> p nk w ts", w=2)[:, :, :, 0:PP])
    # kT2 [128 d-stack, 2w, 4ch, 128 j-slots] block-diag (zero quads persist)
    for hp in range(2):
        nc.scalar.copy(
            kT2[64 * hp:64 * hp + 64, :, :, 64 * hp:64 * hp + PP],
            kT_ps[64 * hp:64 * hp + 64, :, :].rearrange(
                "p nk (w ts) -> p w nk ts", w=2)[:, :, :, 0:PP])

    # 4. v -> psum [128 tok-slots, 512] -> v2 [49, 2w, 8h, 64] bf16
    v_bank = ps.tile([128, 512], FP32, tag="ps")
    v_ps = v_bank[:]
    for kc in range(2):
        nc.tensor.matmul(v_ps[:], fT[:, kc, :], wv_s[:, kc, :],
                         start=(kc == 0), stop=(kc == 1))
    v2 = sb.tile([PP, 2, H, D], BF16, tag="v2")
    for w in range(2):
        nc.vector.tensor_copy(
            v2[:, w, :, :],
            v_ps[64 * w:64 * w + PP, :].rearrange("p (h d) -> p h d", h=H))

    # 5. ST: 2 heads per matmul -> st [128 j-hp-stack, (2w,4ch), 49]
    st_bank = ps.tile([128, 512], FP32, tag="ps")
    st_ps = st_bank[:, 0:392].rearrange("p (w ch t) -> p w ch t", w=2, ch=4)
    for w in range(2):
        for ch in range(4):
            nc.tensor.matmul(st_ps[:, w, ch, :], kT2[:, w, ch, :],
                             qT2[:, ch, w, :], start=True, stop=True)

    # 6. exp (one ACT op per hp) -> expS [49 j, (2w,4ch), 49 i] bf16
    expS = []
    for hp in range(2):
        e = sb.tile([PP, 2, 4, PP], BF16, tag=f"expS{hp}")
        nc.scalar.activation(e[:], st_ps[64 * hp:64 * hp + PP, :, :, :],
                             mybir.ActivationFunctionType.Exp, scale=SCALE)
        expS.append(e)

    # 7. softmax denominators: partition_all_reduce on GPSIMD (idle engine),
    #    then normalize expS in place on DVE (Pool lacks TensorScalarPtr).
    for hp in range(2):
        den = sb.tile([PP, 2, 4, PP], FP32, tag=f"den{hp}")
        nc.gpsimd.partition_all_reduce(den[:], expS[hp][:], channels=PP,
                                       reduce_op=bass_isa.ReduceOp.add)
        nc.vector.reciprocal(den[:], den[:])
        nc.vector.scalar_tensor_tensor(out=expS[hp][:], in0=expS[hp][:],
                                       scalar=1.0, in1=den[:],
                                       op0=mybir.AluOpType.mult,
                                       op1=mybir.AluOpType.mult)

    # 8. av in [d, i] orientation: lhsT=v2 [49j, 64d], rhs=expS [49j, 49i]
    #    -> av [64 d, 49 i] per (h, w), packed in ONE psum bank
    #    [128 (hp col-blocks at 0/64), 2w, 4ch, 49].  This layout IS outT's
    #    layout (inner chunk ch = [64hp+d]), so no output transposes needed.
    av_bank = ps.tile([128, 512], FP32, tag="ps")
    av_ps = av_bank[:, 0:392].rearrange("p (w ch i) -> p w ch i", w=2, ch=4)
    for w in range(2):
        for ch in range(4):
            for hp in range(2):
                h = 2 * ch + hp
                nc.tensor.matmul(
                    av_ps[64 * hp:64 * hp + D, w, ch, :],
                    v2[:, w, h, :],
                    expS[hp][:, w, ch, :],
                    tile_position=(0, 64 * hp), start=True, stop=True)

    # 9. single copy av -> outT [inner(4ch x 128), 2w, 64 tok-slots] bf16
    outT = sb.tile([128, 4, 2, 64], BF16, tag="outT")
    nc.scalar.copy(outT[:, :, :, 0:PP],
                   av_ps[:].rearrange("p w ch i -> p ch w i"))

    # 10. fin = outT.T @ Wo + bo  [128 tok-slots, 256]
    ot_flat = outT[:].rearrange("p ch w i -> p (ch w i)")
    fin_bank = ps.tile([128, 512], FP32, tag="ps")
    fin_ps = fin_bank[:, 0:DIM]
    for nk in range(4):
        nc.tensor.matmul(fin_ps[:], ot_flat[:, 128 * nk:128 * nk + 128],
                         wo_s[:, nk, :], start=(nk == 0), stop=(nk == 3))
    fin = sb.tile([128, DIM], FP32, tag="fin")
    nc.vector.scalar_tensor_tensor(out=fin[:], in0=fin_ps[:], scalar=1.0,
                                   in1=bo_bc[:], op0=mybir.AluOpType.mult,
                                   op1=mybir.AluOpType.add)

    # 12. store (w0 via sync queue, w1 via gpsimd queue)
    for w, eng in ((0, nc.sync), (1, nc.gpsimd)):
        c0 = P * (2 * u + w)
        eng.dma_start(out=out[img, r0:r0 + P, c0:c0 + P, :],
                      in_=fin[64 * w:64 * w + PP, :])


_CACHED = {}


def _get_nc():
    if "nc" not in _CACHED:
        _CACHED["nc"] = build_bass()
    return _CACHED["nc"]


def kernel(fmap, Wq, Wkv, Wo, bo, _trace=False, _trace_kwargs=None):
    fmap = np.ascontiguousarray(fmap, dtype=np.float32)
    nc = _get_nc()
    in_maps = []
    for c in range(NCORES):
        in_maps.append({
            "fmap": fmap[IMGS_PER_CORE * c:IMGS_PER_CORE * (c + 1)],
            "Wq": np.ascontiguousarray(Wq, dtype=np.float32),
            "Wkv": np.ascontiguousarray(Wkv, dtype=np.float32),
            "Wo": np.ascontiguousarray(Wo, dtype=np.float32),
            "bo": np.ascontiguousarray(bo, dtype=np.float32),
        })
    res = run_bass_kernel_spmd(nc, in_maps, core_ids=list(range(NCORES)),
                               trace=_trace, **(_trace_kwargs or {}))
    outs = [r["out"] for r in res.results]
    full = np.concatenate(outs, axis=0)
    if _trace:
        return full, res
    return full


# revision 14
# speedup vs baseline: 2.4747x; 2.4074x over previous
"""Local window attention (7x7 windows, 8 heads, d=64) Trainium2 Bass kernel.

Full inputs in, full outputs out. Data-parallel over batch across 8 cores
(4 images per core). Shapes hardcoded per spec:
  fmap (32, 56, 56, 256) f32, Wq (256,512), Wkv (256,1024), Wo (512,256), bo (256,)

v3: software-pipelined stages (engines execute in-order per queue, and the
tile scheduler follows emission order, so cross-group overlap must be
explicit). Per group g = 2 adjacent-y windows, tokens 64-padded (p=64w+7r+t):

  P(g): 2 input DMAs (one per window, [49,256] <- [7,7,256])
  A(g): 2 PE transposes -> fT; q/k projections (16 mm); qT2/kT2 copies
        (kT2 block-diagonal over 2 heads, zero quadrants persist in
        manually-rotated buffers); v projection (2 mm); v2 copies (ones
        column persists at col 64 for the fused softmax denominator)
  B(g): ST 2-heads-per-matmul (8 mm); exp (2 ACT ops)
  C(g): av+denom (16 mm, N=65, lhsT=expS, rhs=[v|1]); tiny reciprocal of
        the denom column; normalize into out_tok via scalar_tensor_tensor
        with a stride-0 broadcast of 1/den; 4 PE transposes -> outT
  D(g): fin = outT.T @ Wo + bo (4 mm + stt); 2 output DMAs

Emission per iteration i: P(i+1), A(i), B(i-1), C(i-2), D(i-3) — 4 groups
in flight; PSUM allocs/iteration: A:4 B:1 C:3 D:1 = 9 on an 8-bank rotation.
All matmuls bf16 (psum fp32); casts ride the mandatory psum->SBUF copies.
"""

from contextlib import ExitStack

import numpy as np

import concourse.bacc as bacc
import concourse.bass as bass
import concourse.tile as tile
from concourse import mybir
from concourse import bass_isa
from concourse.masks import make_identity
from concourse.bass_utils import run_bass_kernel_spmd

P = 7
PP = 49          # tokens per window
H = 8            # heads
D = 64           # head dim
DIM = 256        # channels
INNER = 512      # h*d
SCALE = D ** -0.5
IMGS_PER_CORE = 4
NCORES = 8
X = 56
NW = X // P      # 8 windows per axis
FP32 = mybir.dt.float32
BF16 = mybir.dt.bfloat16
NROT = 4         # manual rotation depth for persistent tiles


def build_bass(n_imgs=IMGS_PER_CORE):
    nc = bacc.Bacc("TRN2", target_bir_lowering=False, debug=False)

    fm = nc.dram_tensor("fmap", [n_imgs, X, X, DIM], FP32, kind="ExternalInput").ap()
    wq = nc.dram_tensor("Wq", [DIM, INNER], FP32, kind="ExternalInput").ap()
    wkv = nc.dram_tensor("Wkv", [DIM, 2 * INNER], FP32, kind="ExternalInput").ap()
    wo = nc.dram_tensor("Wo", [INNER, DIM], FP32, kind="ExternalInput").ap()
    bo = nc.dram_tensor("bo", [DIM], FP32, kind="ExternalInput").ap()
    out = nc.dram_tensor("out", [n_imgs, X, X, DIM], FP32, kind="ExternalOutput").ap()

    with tile.TileContext(nc) as tc:
        with ExitStack() as ctx:
            build_kernel(ctx, tc, out, fm, wq, wkv, wo, bo, n_imgs)
    nc.compile()
    return nc


def build_kernel(ctx, tc, out, fm, wq, wkv, wo, bo, n_imgs=IMGS_PER_CORE):
    nc = tc.nc
    consts = ctx.enter_context(tc.tile_pool(name="consts", bufs=1))
    sb = ctx.enter_context(tc.tile_pool(name="sb", bufs=4))
    ps = ctx.enter_context(tc.tile_pool(name="ps", bufs=8, space="PSUM"))

    # ---- constants ----
    ident = consts.tile([128, 128], FP32)
    make_identity(nc, ident[:])

    ones = consts.tile([1, 128], FP32)
    nc.gpsimd.memset(ones[:], 1.0)

    def stage_w(dram_ap, shape, name):
        st = sb.tile(shape, FP32, tag="stage")
        nc.sync.dma_start(out=st[:], in_=dram_ap)
        bt = consts.tile(shape, BF16, tag=name)
        nc.vector.tensor_copy(bt[:], st[:])
        return bt

    wq_s = stage_w(wq.rearrange("(kc ck) n -> ck kc n", ck=128), [128, 2, INNER],
                   "wq_s")
    wk_s = stage_w(wkv[:, 0:INNER].rearrange("(kc ck) n -> ck kc n", ck=128),
                   [128, 2, INNER], "wk_s")
    wv_s = stage_w(wkv[:, INNER:2 * INNER].rearrange("(kc ck) n -> ck kc n", ck=128),
                   [128, 2, INNER], "wv_s")
    wo_s = stage_w(wo.rearrange("(kc ck) m -> ck kc m", ck=128), [128, 4, DIM],
                   "wo_s")

    bo_f = consts.tile([1, DIM], FP32)
    nc.sync.dma_start(out=bo_f[:], in_=bo[None, :])
    bb_ps = ps.tile([128, 512], FP32, tag="ps")
    nc.tensor.matmul(bb_ps[:, 0:DIM], ones[0:1, :], bo_f[:], start=True, stop=True)
    bo_bc = consts.tile([128, DIM], FP32)
    nc.scalar.copy(bo_bc[:], bb_ps[:, 0:DIM])

    # persistent rotated buffers: kT2 (zero quadrants), v2 (ones column)
    kT2_bufs, v2_bufs = [], []
    for i in range(NROT):
        t = consts.tile([128, 2, 4, 128], BF16, tag=f"kT2_{i}")
        nc.gpsimd.memset(t[:], 0.0)
        kT2_bufs.append(t)
        v = consts.tile([PP, 2, H, D + 1], BF16, tag=f"v2_{i}")
        nc.gpsimd.memset(v[:, :, :, D:D + 1], 1.0)
        v2_bufs.append(v)

    # ---- software-pipelined main loop ----
    n_groups = n_imgs * NW * (NW // 2)

    def coords(g):
        img, rem = divmod(g, NW * (NW // 2))
        wx, u = divmod(rem, NW // 2)
        return img, wx, u

    st8 = {}  # per-group state carried between stages

    for i in range(n_groups + 3):
        if i == 0:
            stage_p(nc, sb, st8, fm, coords(0), 0)
        if i + 1 < n_groups:
            stage_p(nc, sb, st8, fm, coords(i + 1), i + 1)
        if i < n_groups:
            stage_a(nc, sb, ps, st8, wq_s, wk_s, wv_s, ident,
                    kT2_bufs[i % NROT], v2_bufs[i % NROT], i)
        if 0 <= i - 1 < n_groups:
            stage_b(nc, sb, ps, st8, i - 1)
        if 0 <= i - 2 < n_groups:
            stage_c(nc, sb, ps, st8, ident, i - 2)
        if 0 <= i - 3 < n_groups:
            stage_d(nc, sb, ps, st8, out, wo_s, bo_bc, coords(i - 3), i - 3)


def stage_p(nc, sb, st8, fm, c, g):
    img, wx, u = c
    f_raw = sb.tile([128, DIM], FP32, tag="f_raw")
    for w in range(2):
        c0 = P * (2 * u + w)
        nc.sync.dma_start(out=f_raw[64 * w:64 * w + PP, :],
                          in_=fm[img, P * wx:P * wx + P, c0:c0 + P, :])
    st8[(g, "f_raw")] = f_raw


def stage_a(nc, sb, ps, st8, wq_s, wk_s, wv_s, ident, kT2, v2, g):
    f_raw = st8.pop((g, "f_raw"))

    # transposes -> fT
    fT_bank = ps.tile([128, 512], FP32, tag="ps")
    fT_ps = fT_bank[:, 0:256].rearrange("p (kc t) -> p kc t", kc=2)
    for kc in range(2):
        nc.tensor.transpose(fT_ps[:, kc, :], f_raw[:, 128 * kc:128 * kc + 128],
                            ident[:])
    fT = sb.tile([128, 2, 128], BF16, tag="fT")
    nc.scalar.copy(fT[:], fT_ps[:])

    # q/k projections
    q_bank = ps.tile([128, 512], FP32, tag="ps")
    qT_ps = q_bank[:].rearrange("p (nk t) -> p nk t", nk=4)
    k_bank = ps.tile([128, 512], FP32, tag="ps")
    kT_ps = k_bank[:].rearrange("p (nk t) -> p nk t", nk=4)
    for nk in range(4):
        for kc in range(2):
            nc.tensor.matmul(qT_ps[:, nk, :], wq_s[:, kc, 128 * nk:128 * nk + 128],
                             fT[:, kc, :], start=(kc == 0), stop=(kc == 1))
            nc.tensor.matmul(kT_ps[:, nk, :], wk_s[:, kc, 128 * nk:128 * nk + 128],
                             fT[:, kc, :], start=(kc == 0), stop=(kc == 1))

    qT2 = sb.tile([128, 4, 2, PP], BF16, tag="qT2")
    for hp in range(2):
        nc.vector.tensor_copy(
            qT2[64 * hp:64 * hp + 64, :, :, :],
            qT_ps[64 * hp:64 * hp + 64, :, :].rearrange(
                "p nk (w ts) -> p nk w ts", w=2)[:, :, :, 0:PP])
    for hp in range(2):
        nc.scalar.copy(
            kT2[64 * hp:64 * hp + 64, :, :, 64 * hp:64 * hp + PP],
            kT_ps[64 * hp:64 * hp + 64, :, :].rearrange(
                "p nk (w ts) -> p w nk ts", w=2)[:, :, :, 0:PP])

    # v projection + v2 copies (ones column at D persists)
    v_bank = ps.tile([128, 512], FP32, tag="ps")
    for kc in range(2):
        nc.tensor.matmul(v_bank[:], fT[:, kc, :], wv_s[:, kc, :],
                         start=(kc == 0), stop=(kc == 1))
    nc.scalar.copy(v2[:, 0, :, 0:D],
                   v_bank[0:PP, :].rearrange("p (h d) -> p h d", h=H))
    nc.vector.tensor_copy(v2[:, 1, :, 0:D],
                          v_bank[64:64 + PP, :].rearrange("p (h d) -> p h d", h=H))

    st8[(g, "fT")] = fT
    st8[(g, "qT2")] = qT2
    st8[(g, "kT2")] = kT2
    st8[(g, "v2")] = v2


def stage_b(nc, sb, ps, st8, g):
    qT2 = st8.pop((g, "qT2"))
    kT2 = st8.pop((g, "kT2"))
    st8.pop((g, "fT"))

    st_bank = ps.tile([128, 512], FP32, tag="ps")
    st_ps = st_bank[:, 0:392].rearrange("p (w ch t) -> p w ch t", w=2, ch=4)
    for w in range(2):
        for ch in range(4):
            nc.tensor.matmul(st_ps[:, w, ch, :], kT2[:, w, ch, :],
                             qT2[:, ch, w, :], start=True, stop=True)

    expS = []
    for hp in range(2):
        e = sb.tile([PP, 2, 4, PP], BF16, tag=f"expS{hp}")
        nc.scalar.activation(e[:], st_ps[64 * hp:64 * hp + PP, :, :, :],
                             mybir.ActivationFunctionType.Exp, scale=SCALE)
        expS.append(e)
    st8[(g, "expS")] = expS


def stage_c(nc, sb, ps, st8, ident, g):
    expS = st8.pop((g, "expS"))
    v2 = st8.pop((g, "v2"))

    # av + fused denominator: [49 i, 65] per (h, w); one psum bank per w
    av_banks = []
    for w in range(2):
        avb = ps.tile([128, 512], FP32, tag="ps")
        av = avb[:, 0:260].rearrange("p (ch e) -> p ch e", ch=4)
        av_banks.append(av)
        for ch in range(4):
            for hp in range(2):
                h = 2 * ch + hp
                nc.tensor.matmul(
                    av[64 * hp:64 * hp + PP, ch, :],
                    expS[hp][:, w, ch, :],
                    v2[:, w, h, :],
                    tile_position=(0, 64 * hp), start=True, stop=True)

    # tiny reciprocal of denom column; normalize into out_tok
    out_tok = sb.tile([128, 4, 2, D], FP32, tag="out_tok")  # free = (ch, hp, d)
    for w in range(2):
        av = av_banks[w]
        recd = sb.tile([128, 4, 1], FP32, tag=f"recd{w}")
        nc.vector.reciprocal(recd[0:113, :, :], av[0:113, :, D:D + 1])
        for hp in range(2):
            nc.vector.scalar_tensor_tensor(
                out=out_tok[64 * w:64 * w + PP, :, hp, :],
                in0=av[64 * hp:64 * hp + PP, :, 0:D],
                scalar=1.0,
                in1=recd[64 * hp:64 * hp + PP, :, 0:1].broadcast_to((PP, 4, D)),
                op0=mybir.AluOpType.mult, op1=mybir.AluOpType.mult)

    # transpose out_tok -> outT
    ot_flat = out_tok[:].rearrange("p ch hp d -> p (ch hp d)")
    ot_bank = ps.tile([128, 512], FP32, tag="ps")
    ot_ps = ot_bank[:].rearrange("p (nk t) -> p nk t", nk=4)
    for nk in range(4):
        nc.tensor.transpose(ot_ps[:, nk, :], ot_flat[:, 128 * nk:128 * nk + 128],
                            ident[:])
    outT = sb.tile([128, 4, 128], BF16, tag="outT")
    nc.scalar.copy(outT[:], ot_ps[:])
    st8[(g, "outT")] = outT


def stage_d(nc, sb, ps, st8, out, wo_s, bo_bc, c, g):
    img, wx, u = c
    outT = st8.pop((g, "outT"))

    fin_bank = ps.tile([128, 512], FP32, tag="ps")
    fin_ps = fin_bank[:, 0:DIM]
    for nk in range(4):
        nc.tensor.matmul(fin_ps[:], outT[:, nk, :], wo_s[:, nk, :],
                         start=(nk == 0), stop=(nk == 3))
    fin = sb.tile([128, DIM], FP32, tag="fin")
    nc.vector.scalar_tensor_tensor(out=fin[:], in0=fin_ps[:], scalar=1.0,
                                   in1=bo_bc[:], op0=mybir.AluOpType.mult,
                                   op1=mybir.AluOpType.add)

    for w, eng in ((0, nc.sync), (1, nc.gpsimd)):
        c0 = P * (2 * u + w)
        eng.dma_start(out=out[img, P * wx:P * wx + P, c0:c0 + P, :],
                      in_=fin[64 * w:64 * w + PP, :])


_CACHED = {}


def _get_nc():
    if "nc" not in _CACHED:
        _CACHED["nc"] = build_bass()
    return _CACHED["nc"]


def kernel(fmap, Wq, Wkv, Wo, bo, _trace=False, _trace_kwargs=None):
    fmap = np.ascontiguousarray(fmap, dtype=np.float32)
    nc = _get_nc()
    in_maps = []
    for c in range(NCORES):
        in_maps.append({
            "fmap": fmap[IMGS_PER_CORE * c:IMGS_PER_CORE * (c + 1)],
            "Wq": np.ascontiguousarray(Wq, dtype=np.float32),
            "Wkv": np.ascontiguousarray(Wkv, dtype=np.float32),
            "Wo": np.ascontiguousarray(Wo, dtype=np.float32),
            "bo": np.ascontiguousarray(bo, dtype=np.float32),
        })
    res = run_bass_kernel_spmd(nc, in_maps, core_ids=list(range(NCORES)),
                               trace=_trace, **(_trace_kwargs or {}))
    outs = [r["out"] for r in res.results]
    full = np.concatenate(outs, axis=0)
    if _trace:
        return full, res
    return full


# revision 17
# speedup vs baseline: 2.7033x; 1.0924x over previous
"""Local window attention (7x7 windows, 8 heads, d=64) Trainium2 Bass kernel.

Full inputs in, full outputs out. Data-parallel over batch across 8 cores
(4 images per core). Shapes hardcoded per spec:
  fmap (32, 56, 56, 256) f32, Wq (256,512), Wkv (256,1024), Wo (512,256), bo (256,)

v3: software-pipelined stages (engines execute in-order per queue, and the
tile scheduler follows emission order, so cross-group overlap must be
explicit). Per group g = 2 adjacent-y windows, tokens 64-padded (p=64w+7r+t):

  P(g): 2 input DMAs (one per window, [49,256] <- [7,7,256])
  A(g): 2 PE transposes -> fT; q/k projections (16 mm); qT2/kT2 copies
        (kT2 block-diagonal over 2 heads, zero quadrants persist in
        manually-rotated buffers); v projection (2 mm); v2 copies (ones
        column persists at col 64 for the fused softmax denominator)
  B(g): ST 2-heads-per-matmul (8 mm); exp (2 ACT ops)
  C(g): av+denom (16 mm, N=65, lhsT=expS, rhs=[v|1]); tiny reciprocal of
        the denom column; normalize into out_tok via scalar_tensor_tensor
        with a stride-0 broadcast of 1/den; 4 PE transposes -> outT
  D(g): fin = outT.T @ Wo + bo (4 mm + stt); 2 output DMAs

Emission per iteration i: P(i+1), A(i), B(i-1), C(i-2), D(i-3) — 4 groups
in flight; PSUM allocs/iteration: A:4 B:1 C:3 D:1 = 9 on an 8-bank rotation.
All matmuls bf16 (psum fp32); casts ride the mandatory psum->SBUF copies.
"""

from contextlib import ExitStack

import numpy as np

import concourse.bacc as bacc
import concourse.bass as bass
import concourse.tile as tile
from concourse import mybir
from concourse import bass_isa
from concourse.masks import make_identity
from concourse.bass_utils import run_bass_kernel_spmd

P = 7
PP = 49          # tokens per window
H = 8            # heads
D = 64           # head dim
DIM = 256        # channels
INNER = 512      # h*d
SCALE = D ** -0.5
IMGS_PER_CORE = 4
NCORES = 8
X = 56
NW = X // P      # 8 windows per axis
FP32 = mybir.dt.float32
BF16 = mybir.dt.bfloat16
NROT = 4         # manual rotation depth for persistent tiles


def build_bass(n_imgs=IMGS_PER_CORE):
    nc = bacc.Bacc("TRN2", target_bir_lowering=False, debug=False)

    fm = nc.dram_tensor("fmap", [n_imgs, X, X, DIM], FP32, kind="ExternalInput").ap()
    wq = nc.dram_tensor("Wq", [DIM, INNER], FP32, kind="ExternalInput").ap()
    wkv = nc.dram_tensor("Wkv", [DIM, 2 * INNER], FP32, kind="ExternalInput").ap()
    wo = nc.dram_tensor("Wo", [INNER, DIM], FP32, kind="ExternalInput").ap()
    bo = nc.dram_tensor("bo", [DIM], FP32, kind="ExternalInput").ap()
    out = nc.dram_tensor("out", [n_imgs, X, X, DIM], FP32, kind="ExternalOutput").ap()

    with tile.TileContext(nc) as tc:
        with ExitStack() as ctx:
            build_kernel(ctx, tc, out, fm, wq, wkv, wo, bo, n_imgs)
    nc.compile()
    return nc


def build_kernel(ctx, tc, out, fm, wq, wkv, wo, bo, n_imgs=IMGS_PER_CORE):
    nc = tc.nc
    consts = ctx.enter_context(tc.tile_pool(name="consts", bufs=1))
    sb = ctx.enter_context(tc.tile_pool(name="sb", bufs=4))
    ps = ctx.enter_context(tc.tile_pool(name="ps", bufs=8, space="PSUM"))

    # ---- constants ----
    ident = consts.tile([128, 128], FP32)
    make_identity(nc, ident[:])

    ones = consts.tile([1, 128], FP32)
    nc.gpsimd.memset(ones[:], 1.0)
    identb = consts.tile([128, 128], BF16)
    nc.vector.tensor_copy(identb[:], ident[:])

    def stage_w(dram_ap, shape, name):
        st = sb.tile(shape, FP32, tag="stage")
        nc.sync.dma_start(out=st[:], in_=dram_ap)
        bt = consts.tile(shape, BF16, tag=name)
        nc.vector.tensor_copy(bt[:], st[:])
        return bt

    wq_s = stage_w(wq.rearrange("(kc ck) n -> ck kc n", ck=128), [128, 2, INNER],
                   "wq_s")
    wk_s = stage_w(wkv[:, 0:INNER].rearrange("(kc ck) n -> ck kc n", ck=128),
                   [128, 2, INNER], "wk_s")
    wv_s = stage_w(wkv[:, INNER:2 * INNER].rearrange("(kc ck) n -> ck kc n", ck=128),
                   [128, 2, INNER], "wv_s")
    wo_s = stage_w(wo.rearrange("(kc ck) m -> ck kc m", ck=128), [128, 4, DIM],
                   "wo_s")

    bo_f = consts.tile([1, DIM], FP32)
    nc.sync.dma_start(out=bo_f[:], in_=bo[None, :])
    bb_ps = ps.tile([128, 512], FP32, tag="ps")
    nc.tensor.matmul(bb_ps[:, 0:DIM], ones[0:1, :], bo_f[:], start=True, stop=True)
    bo_bc = consts.tile([128, DIM], FP32)
    nc.scalar.copy(bo_bc[:], bb_ps[:, 0:DIM])

    # persistent rotated buffers: kT2 (zero quadrants), v2 (ones column)
    kT2_bufs, v2_bufs = [], []
    for i in range(NROT):
        t = consts.tile([128, 2, 4, 128], BF16, tag=f"kT2_{i}")
        nc.gpsimd.memset(t[:], 0.0)
        kT2_bufs.append(t)
        v = consts.tile([PP, 2, H, D + 1], BF16, tag=f"v2_{i}")
        nc.gpsimd.memset(v[:, :, :, D:D + 1], 1.0)
        v2_bufs.append(v)

    # ---- software-pipelined main loop ----
    n_groups = n_imgs * NW * (NW // 2)

    def coords(g):
        img, rem = divmod(g, NW * (NW // 2))
        wx, u = divmod(rem, NW // 2)
        return img, wx, u

    st8 = {}  # per-group state carried between stages

    for i in range(n_groups + 3):
        if i == 0:
            stage_p(nc, sb, st8, fm, coords(0), 0)
        if i + 1 < n_groups:
            stage_p(nc, sb, st8, fm, coords(i + 1), i + 1)
        if i < n_groups:
            stage_a(nc, sb, ps, st8, wq_s, wk_s, wv_s, ident,
                    kT2_bufs[i % NROT], v2_bufs[i % NROT], i)
        if 0 <= i - 1 < n_groups:
            stage_b(nc, sb, ps, st8, i - 1)
        if 0 <= i - 2 < n_groups:
            stage_c(nc, sb, ps, st8, identb, i - 2)
        if 0 <= i - 3 < n_groups:
            stage_d(nc, sb, ps, st8, out, wo_s, bo_bc, coords(i - 3), i - 3)


def stage_p(nc, sb, st8, fm, c, g):
    img, wx, u = c
    f_raw = sb.tile([128, DIM], FP32, tag="f_raw")
    for w, eng in ((0, nc.gpsimd), (1, nc.sync)):
        c0 = P * (2 * u + w)
        eng.dma_start(out=f_raw[64 * w:64 * w + PP, :],
                      in_=fm[img, P * wx:P * wx + P, c0:c0 + P, :])
    st8[(g, "f_raw")] = f_raw


def stage_a(nc, sb, ps, st8, wq_s, wk_s, wv_s, ident, kT2, v2, g):
    f_raw = st8.pop((g, "f_raw"))

    # transposes -> fT
    fT_bank = ps.tile([128, 512], FP32, tag="ps")
    fT_ps = fT_bank[:, 0:256].rearrange("p (kc t) -> p kc t", kc=2)
    for kc in range(2):
        nc.tensor.transpose(fT_ps[:, kc, 0:113], f_raw[0:113, 128 * kc:128 * kc + 128],
                            ident[0:113, 0:113])
    fT = sb.tile([128, 2, 128], BF16, tag="fT")
    nc.scalar.copy(fT[:], fT_ps[:])

    # q/k projections
    fT_c = fT[:].rearrange("p kc (w ts) -> p kc w ts", w=2)[:, :, :, 0:PP]
    q_bank = ps.tile([128, 512], FP32, tag="ps")
    qT_ps = q_bank[:, 0:392].rearrange("p (nk w t) -> p nk w t", nk=4, w=2)
    k_bank = ps.tile([128, 512], FP32, tag="ps")
    kT_ps = k_bank[:, 0:392].rearrange("p (nk w t) -> p nk w t", nk=4, w=2)
    for nk in range(4):
        for kc in range(2):
            nc.tensor.matmul(qT_ps[:, nk, :, :], wq_s[:, kc, 128 * nk:128 * nk + 128],
                             fT_c[:, kc, :, :], start=(kc == 0), stop=(kc == 1))
            nc.tensor.matmul(kT_ps[:, nk, :, :], wk_s[:, kc, 128 * nk:128 * nk + 128],
                             fT_c[:, kc, :, :], start=(kc == 0), stop=(kc == 1))

    qT2 = sb.tile([128, 4, 2, PP], BF16, tag="qT2")
    for hp in range(2):
        nc.vector.tensor_copy(
            qT2[64 * hp:64 * hp + 64, :, :, :],
            qT_ps[64 * hp:64 * hp + 64, :, :, :])
    for hp in range(2):
        nc.scalar.copy(
            kT2[64 * hp:64 * hp + 64, :, :, 64 * hp:64 * hp + PP],
            kT_ps[64 * hp:64 * hp + 64, :, :, :].rearrange(
                "p nk w ts -> p w nk ts"))

    # v projection + v2 copies (ones column at D persists)
    v_bank = ps.tile([128, 512], FP32, tag="ps")
    for kc in range(2):
        nc.tensor.matmul(v_bank[:], fT[:, kc, :], wv_s[:, kc, :],
                         start=(kc == 0), stop=(kc == 1))
    nc.scalar.copy(v2[:, 0, :, 0:D],
                   v_bank[0:PP, :].rearrange("p (h d) -> p h d", h=H))
    nc.vector.tensor_copy(v2[:, 1, :, 0:D],
                          v_bank[64:64 + PP, :].rearrange("p (h d) -> p h d", h=H))

    st8[(g, "fT")] = fT
    st8[(g, "qT2")] = qT2
    st8[(g, "kT2")] = kT2
    st8[(g, "v2")] = v2


def stage_b(nc, sb, ps, st8, g):
    qT2 = st8.pop((g, "qT2"))
    kT2 = st8.pop((g, "kT2"))
    st8.pop((g, "fT"))

    st_bank = ps.tile([128, 512], FP32, tag="ps")
    st_ps = st_bank[:, 0:392].rearrange("p (w ch t) -> p w ch t", w=2, ch=4)
    for w in range(2):
        for ch in range(4):
            nc.tensor.matmul(st_ps[:, w, ch, :], kT2[:, w, ch, :],
                             qT2[:, ch, w, :], start=True, stop=True)

    expS = []
    for hp in range(2):
        e = sb.tile([PP, 2, 4, PP], BF16, tag=f"expS{hp}")
        nc.scalar.activation(e[:], st_ps[64 * hp:64 * hp + PP, :, :, :],
                             mybir.ActivationFunctionType.Exp, scale=SCALE)
        expS.append(e)
    st8[(g, "expS")] = expS


def stage_c(nc, sb, ps, st8, identb, g):
    expS = st8.pop((g, "expS"))
    v2 = st8.pop((g, "v2"))

    # av + fused denominator: [49 i, 65] per (h, w); one psum bank per w
    av_banks = []
    for w in range(2):
        avb = ps.tile([128, 512], FP32, tag="ps")
        av = avb[:, 0:260].rearrange("p (ch e) -> p ch e", ch=4)
        av_banks.append(av)
        for ch in range(4):
            for hp in range(2):
                h = 2 * ch + hp
                nc.tensor.matmul(
                    av[64 * hp:64 * hp + PP, ch, :],
                    expS[hp][:, w, ch, :],
                    v2[:, w, h, :],
                    tile_position=(0, 64 * hp), start=True, stop=True)

    # tiny reciprocal of denom column; normalize into out_tok
    out_tok = sb.tile([128, 4, 2, D], BF16, tag="out_tok")  # free = (ch, hp, d)
    for w in range(2):
        av = av_banks[w]
        recd = sb.tile([128, 4, 1], FP32, tag=f"recd{w}")
        nc.vector.reciprocal(recd[0:113, :, :], av[0:113, :, D:D + 1])
        for hp in range(2):
            nc.vector.scalar_tensor_tensor(
                out=out_tok[64 * w:64 * w + PP, :, hp, :],
                in0=av[64 * hp:64 * hp + PP, :, 0:D],
                scalar=1.0,
                in1=recd[64 * hp:64 * hp + PP, :, 0:1].broadcast_to((PP, 4, D)),
                op0=mybir.AluOpType.mult, op1=mybir.AluOpType.mult)

    # transpose out_tok -> outT
    ot_flat = out_tok[:].rearrange("p ch hp d -> p (ch hp d)")
    ot_bank = ps.tile([128, 1024], BF16, tag="ps")
    ot_ps = ot_bank[:, 0:512].rearrange("p (nk t) -> p nk t", nk=4)
    for nk in range(4):
        nc.tensor.transpose(ot_ps[:, nk, 0:113], ot_flat[0:113, 128 * nk:128 * nk + 128],
                            identb[0:113, 0:113])
    outT = sb.tile([128, 4, 128], BF16, tag="outT")
    nc.scalar.copy(outT[:], ot_ps[:])
    st8[(g, "outT")] = outT


def stage_d(nc, sb, ps, st8, out, wo_s, bo_bc, c, g):
    img, wx, u = c
    outT = st8.pop((g, "outT"))

    fin_bank = ps.tile([128, 512], FP32, tag="ps")
    fin_ps = fin_bank[:, 0:DIM]
    for nk in range(4):
        nc.tensor.matmul(fin_ps[:], outT[:, nk, :], wo_s[:, nk, :],
                         start=(nk == 0), stop=(nk == 3))
    fin = sb.tile([128, DIM], FP32, tag="fin")
    nc.vector.scalar_tensor_tensor(out=fin[:], in0=fin_ps[:], scalar=1.0,
                                   in1=bo_bc[:], op0=mybir.AluOpType.mult,
                                   op1=mybir.AluOpType.add)

    for w, eng in ((0, nc.sync), (1, nc.gpsimd)):
        c0 = P * (2 * u + w)
        eng.dma_start(out=out[img, P * wx:P * wx + P, c0:c0 + P, :],
                      in_=fin[64 * w:64 * w + PP, :])


_CACHED = {}


def _get_nc():
    if "nc" not in _CACHED:
        _CACHED["nc"] = build_bass()
    return _CACHED["nc"]


def kernel(fmap, Wq, Wkv, Wo, bo, _trace=False, _trace_kwargs=None):
    fmap = np.ascontiguousarray(fmap, dtype=np.float32)
    nc = _get_nc()
    in_maps = []
    for c in range(NCORES):
        in_maps.append({
            "fmap": fmap[IMGS_PER_CORE * c:IMGS_PER_CORE * (c + 1)],
            "Wq": np.ascontiguousarray(Wq, dtype=np.float32),
            "Wkv": np.ascontiguousarray(Wkv, dtype=np.float32),
            "Wo": np.ascontiguousarray(Wo, dtype=np.float32),
            "bo": np.ascontiguousarray(bo, dtype=np.float32),
        })
    res = run_bass_kernel_spmd(nc, in_maps, core_ids=list(range(NCORES)),
                               trace=_trace, **(_trace_kwargs or {}))
    outs = [r["out"] for r in res.results]
    full = np.concatenate(outs, axis=0)
    if _trace:
        return full, res
    return full


# revision 18
# speedup vs baseline: 3.3200x; 1.2281x over previous
"""Local window attention (7x7 windows, 8 heads, d=64) Trainium2 Bass kernel.

Full inputs in, full outputs out. Data-parallel over batch across 8 cores
(4 images per core). Shapes hardcoded per spec:
  fmap (32, 56, 56, 256) f32, Wq (256,512), Wkv (256,1024), Wo (512,256), bo (256,)

v3: software-pipelined stages (engines execute in-order per queue, and the
tile scheduler follows emission order, so cross-group overlap must be
explicit). Per group g = 2 adjacent-y windows, tokens 64-padded (p=64w+7r+t):

  P(g): 2 input DMAs (one per window, [49,256] <- [7,7,256])
  A(g): 2 PE transposes -> fT; q/k projections (16 mm); qT2/kT2 copies
        (kT2 block-diagonal over 2 heads, zero quadrants persist in
        manually-rotated buffers); v projection (2 mm); v2 copies (ones
        column persists at col 64 for the fused softmax denominator)
  B(g): ST 2-heads-per-matmul (8 mm); exp (2 ACT ops)
  C(g): av+denom (16 mm, N=65, lhsT=expS, rhs=[v|1]); tiny reciprocal of
        the denom column; normalize into out_tok via scalar_tensor_tensor
        with a stride-0 broadcast of 1/den; 4 PE transposes -> outT
  D(g): fin = outT.T @ Wo + bo (4 mm + stt); 2 output DMAs

Emission per iteration i: P(i+1), A(i), B(i-1), C(i-2), D(i-3) — 4 groups
in flight; PSUM allocs/iteration: A:4 B:1 C:3 D:1 = 9 on an 8-bank rotation.
All matmuls bf16 (psum fp32); casts ride the mandatory psum->SBUF copies.
"""

from contextlib import ExitStack

import numpy as np

import concourse.bacc as bacc
import concourse.bass as bass
import concourse.tile as tile
from concourse import mybir
from concourse import bass_isa
from concourse.masks import make_identity
from concourse.bass_utils import run_bass_kernel_spmd

P = 7
PP = 49          # tokens per window
H = 8            # heads
D = 64           # head dim
DIM = 256        # channels
INNER = 512      # h*d
SCALE = D ** -0.5
IMGS_PER_CORE = 4
NCORES = 8
X = 56
NW = X // P      # 8 windows per axis
FP32 = mybir.dt.float32
BF16 = mybir.dt.bfloat16
NROT = 4         # manual rotation depth for persistent tiles


def build_bass(n_imgs=IMGS_PER_CORE):
    nc = bacc.Bacc("TRN2", target_bir_lowering=False, debug=False)

    fm = nc.dram_tensor("fmap", [n_imgs, X, X, DIM], FP32, kind="ExternalInput").ap()
    wq = nc.dram_tensor("Wq", [DIM, INNER], FP32, kind="ExternalInput").ap()
    wkv = nc.dram_tensor("Wkv", [DIM, 2 * INNER], FP32, kind="ExternalInput").ap()
    wo = nc.dram_tensor("Wo", [INNER, DIM], FP32, kind="ExternalInput").ap()
    bo = nc.dram_tensor("bo", [DIM], FP32, kind="ExternalInput").ap()
    out = nc.dram_tensor("out", [n_imgs, X, X, DIM], FP32, kind="ExternalOutput").ap()

    with tile.TileContext(nc) as tc:
        with ExitStack() as ctx:
            build_kernel(ctx, tc, out, fm, wq, wkv, wo, bo, n_imgs)
    nc.compile()
    return nc


def build_kernel(ctx, tc, out, fm, wq, wkv, wo, bo, n_imgs=IMGS_PER_CORE):
    nc = tc.nc
    consts = ctx.enter_context(tc.tile_pool(name="consts", bufs=1))
    sb = ctx.enter_context(tc.tile_pool(name="sb", bufs=4))
    ps = ctx.enter_context(tc.tile_pool(name="ps", bufs=8, space="PSUM"))

    # ---- constants ----
    ident = consts.tile([128, 128], FP32)
    make_identity(nc, ident[:])

    ones = consts.tile([1, 128], FP32)
    nc.gpsimd.memset(ones[:], 1.0)
    identb = consts.tile([128, 128], BF16)
    nc.vector.tensor_copy(identb[:], ident[:])

    def stage_w(dram_ap, shape, name):
        st = sb.tile(shape, FP32, tag="stage")
        nc.sync.dma_start(out=st[:], in_=dram_ap)
        bt = consts.tile(shape, BF16, tag=name)
        nc.vector.tensor_copy(bt[:], st[:])
        return bt

    wq_s = stage_w(wq.rearrange("(kc ck) n -> ck kc n", ck=128), [128, 2, INNER],
                   "wq_s")
    wk_s = stage_w(wkv[:, 0:INNER].rearrange("(kc ck) n -> ck kc n", ck=128),
                   [128, 2, INNER], "wk_s")
    wv_s = stage_w(wkv[:, INNER:2 * INNER].rearrange("(kc ck) n -> ck kc n", ck=128),
                   [128, 2, INNER], "wv_s")
    wo_s = stage_w(wo.rearrange("(kc ck) m -> ck kc m", ck=128), [128, 4, DIM],
                   "wo_s")

    bo_f = consts.tile([1, DIM], FP32)
    nc.sync.dma_start(out=bo_f[:], in_=bo[None, :])
    bb_ps = ps.tile([128, 512], FP32, tag="ps")
    nc.tensor.matmul(bb_ps[:, 0:DIM], ones[0:1, :], bo_f[:], start=True, stop=True)
    bo_bc = consts.tile([128, DIM], FP32)
    nc.scalar.copy(bo_bc[:], bb_ps[:, 0:DIM])

    # persistent rotated buffers: kT2 (zero quadrants), v2 (ones column)
    kT2_bufs, v2_bufs = [], []
    for i in range(NROT):
        t = consts.tile([128, 2, 4, 128], BF16, tag=f"kT2_{i}")
        nc.gpsimd.memset(t[:], 0.0)
        kT2_bufs.append(t)
        v = consts.tile([PP, 2, H, D + 1], BF16, tag=f"v2_{i}")
        nc.gpsimd.memset(v[:, :, :, D:D + 1], 1.0)
        v2_bufs.append(v)

    # ---- software-pipelined main loop (v5: per-engine readiness order) ----
    # Group g's stages: P at iter g-1, A at g, B at g+1, C1 at g+2,
    # C2 at g+3, D at g+4. Within an iteration, ops are emitted so that
    # every engine's in-order queue meets its dependencies without stalling:
    # PE runs [transp(i), st(i-1), av(i-2), otT(i-3), fin(i-4), qk+v(i)],
    # ACT runs [fT(i), exp(i-1), outT(i-3), kT2(i)],
    # DVE runs [recd+norm(i-2), fin-stt(i-4), qT2(i), v2(i)].
    n_groups = n_imgs * NW * (NW // 2)

    def coords(g):
        img, rem = divmod(g, NW * (NW // 2))
        wx, u = divmod(rem, NW // 2)
        return img, wx, u

    s = {}  # cross-stage state, keyed (group, name)

    def live(g):
        return 0 <= g < n_groups

    for i in range(n_groups + 5):
        if i == 0:
            em_dma_in(nc, sb, s, fm, coords(0), 0)
        if live(i + 1):
            em_dma_in(nc, sb, s, fm, coords(i + 1), i + 1)
        if live(i):
            em_pe_transp(nc, sb, ps, s, ident, i)
        if live(i - 1):
            em_pe_st(nc, ps, s, i - 1)
        if live(i - 2):
            em_pe_av(nc, ps, s, i - 2)
        if live(i - 3):
            em_pe_ot(nc, ps, s, identb, i - 3)
        if live(i - 4):
            em_pe_fin(nc, ps, s, wo_s, i - 4)
        if live(i):
            em_act_ft(nc, sb, s, i)
        if live(i - 1):
            em_act_exp(nc, sb, s, i - 1)
        if live(i - 3):
            em_act_outT(nc, sb, s, i - 3)
        if live(i - 2):
            em_dve_norm(nc, sb, s, i - 2)
        if live(i - 4):
            em_dve_fin(nc, sb, s, bo_bc, i - 4)
        if live(i):
            em_pe_qkv(nc, ps, s, wq_s, wk_s, wv_s, i)
            em_copies_tail(nc, sb, s, kT2_bufs[i % NROT], v2_bufs[i % NROT], i)
        if live(i - 4):
            em_dma_out(nc, s, out, coords(i - 4), i - 4)


def em_dma_in(nc, sb, s, fm, c, g):
    img, wx, u = c
    f_raw = sb.tile([128, DIM], FP32, tag="f_raw")
    for w, eng in ((0, nc.gpsimd), (1, nc.sync)):
        c0 = P * (2 * u + w)
        eng.dma_start(out=f_raw[64 * w:64 * w + PP, :],
                      in_=fm[img, P * wx:P * wx + P, c0:c0 + P, :])
    s[(g, "f_raw")] = f_raw


def em_pe_transp(nc, sb, ps, s, ident, g):
    f_raw = s.pop((g, "f_raw"))
    fT_bank = ps.tile([128, 512], FP32, tag="ps")
    fT_ps = fT_bank[:, 0:256].rearrange("p (kc t) -> p kc t", kc=2)
    for kc in range(2):
        nc.tensor.transpose(fT_ps[:, kc, 0:113],
                            f_raw[0:113, 128 * kc:128 * kc + 128],
                            ident[0:113, 0:113])
    s[(g, "fT_ps")] = fT_ps


def em_pe_st(nc, ps, s, g):
    qT2 = s.pop((g, "qT2"))
    kT2 = s.pop((g, "kT2"))
    st_bank = ps.tile([128, 512], FP32, tag="ps")
    st_ps = st_bank[:, 0:392].rearrange("p (w ch t) -> p w ch t", w=2, ch=4)
    for w in range(2):
        for ch in range(4):
            nc.tensor.matmul(st_ps[:, w, ch, :], kT2[:, w, ch, :],
                             qT2[:, ch, w, :], start=True, stop=True)
    s[(g, "st_ps")] = st_ps


def em_pe_av(nc, ps, s, g):
    expS = s.pop((g, "expS"))
    v2 = s.pop((g, "v2"))
    av_banks = []
    for w in range(2):
        avb = ps.tile([128, 512], FP32, tag="ps")
        av = avb[:, 0:260].rearrange("p (ch e) -> p ch e", ch=4)
        av_banks.append(av)
        for ch in range(4):
            for hp in range(2):
                h = 2 * ch + hp
                nc.tensor.matmul(
                    av[64 * hp:64 * hp + PP, ch, :],
                    expS[hp][:, w, ch, :],
                    v2[:, w, h, :],
                    tile_position=(0, 64 * hp), start=True, stop=True)
    s[(g, "av")] = av_banks


def em_pe_ot(nc, ps, s, identb, g):
    out_tok = s.pop((g, "out_tok"))
    ot_flat = out_tok[:].rearrange("p ch hp d -> p (ch hp d)")
    ot_bank = ps.tile([128, 1024], BF16, tag="ps")
    ot_ps = ot_bank[:, 0:512].rearrange("p (nk t) -> p nk t", nk=4)
    for nk in range(4):
        nc.tensor.transpose(ot_ps[:, nk, 0:113],
                            ot_flat[0:113, 128 * nk:128 * nk + 128],
                            identb[0:113, 0:113])
    s[(g, "ot_ps")] = ot_ps


def em_pe_fin(nc, ps, s, wo_s, g):
    outT = s.pop((g, "outT"))
    fin_bank = ps.tile([128, 512], FP32, tag="ps")
    fin_ps = fin_bank[:, 0:DIM]
    for nk in range(4):
        nc.tensor.matmul(fin_ps[:], outT[:, nk, :], wo_s[:, nk, :],
                         start=(nk == 0), stop=(nk == 3))
    s[(g, "fin_ps")] = fin_ps


def em_act_ft(nc, sb, s, g):
    fT_ps = s.pop((g, "fT_ps"))
    fT = sb.tile([128, 2, 128], BF16, tag="fT")
    nc.scalar.copy(fT[:], fT_ps[:])
    s[(g, "fT")] = fT


def em_act_exp(nc, sb, s, g):
    st_ps = s.pop((g, "st_ps"))
    expS = []
    for hp in range(2):
        e = sb.tile([PP, 2, 4, PP], BF16, tag=f"expS{hp}")
        nc.scalar.activation(e[:], st_ps[64 * hp:64 * hp + PP, :, :, :],
                             mybir.ActivationFunctionType.Exp, scale=SCALE)
        expS.append(e)
    s[(g, "expS")] = expS


def em_act_outT(nc, sb, s, g):
    ot_ps = s.pop((g, "ot_ps"))
    outT = sb.tile([128, 4, 128], BF16, tag="outT")
    nc.scalar.copy(outT[:], ot_ps[:])
    s[(g, "outT")] = outT


def em_dve_norm(nc, sb, s, g):
    av_banks = s.pop((g, "av"))
    out_tok = sb.tile([128, 4, 2, D], BF16, tag="out_tok")  # free = (ch, hp, d)
    for w in range(2):
        av = av_banks[w]
        recd = sb.tile([128, 4, 1], FP32, tag=f"recd{w}")
        nc.vector.reciprocal(recd[0:113, :, :], av[0:113, :, D:D + 1])
        for hp in range(2):
            nc.vector.scalar_tensor_tensor(
                out=out_tok[64 * w:64 * w + PP, :, hp, :],
                in0=av[64 * hp:64 * hp + PP, :, 0:D],
                scalar=1.0,
                in1=recd[64 * hp:64 * hp + PP, :, 0:1].broadcast_to((PP, 4, D)),
                op0=mybir.AluOpType.mult, op1=mybir.AluOpType.mult)
    s[(g, "out_tok")] = out_tok


def em_dve_fin(nc, sb, s, bo_bc, g):
    fin_ps = s.pop((g, "fin_ps"))
    fin = sb.tile([128, DIM], FP32, tag="fin")
    nc.vector.scalar_tensor_tensor(out=fin[:], in0=fin_ps[:], scalar=1.0,
                                   in1=bo_bc[:], op0=mybir.AluOpType.mult,
                                   op1=mybir.AluOpType.add)
    s[(g, "fin")] = fin


def em_pe_qkv(nc, ps, s, wq_s, wk_s, wv_s, g):
    fT = s.pop((g, "fT"))
    fT_c = fT[:].rearrange("p kc (w ts) -> p kc w ts", w=2)[:, :, :, 0:PP]
    q_bank = ps.tile([128, 512], FP32, tag="ps")
    qT_ps = q_bank[:, 0:392].rearrange("p (nk w t) -> p nk w t", nk=4, w=2)
    k_bank = ps.tile([128, 512], FP32, tag="ps")
    kT_ps = k_bank[:, 0:392].rearrange("p (nk w t) -> p nk w t", nk=4, w=2)
    for nk in range(4):
        for kc in range(2):
            nc.tensor.matmul(qT_ps[:, nk, :, :],
                             wq_s[:, kc, 128 * nk:128 * nk + 128],
                             fT_c[:, kc, :, :], start=(kc == 0), stop=(kc == 1))
            nc.tensor.matmul(kT_ps[:, nk, :, :],
                             wk_s[:, kc, 128 * nk:128 * nk + 128],
                             fT_c[:, kc, :, :], start=(kc == 0), stop=(kc == 1))
    v_bank = ps.tile([128, 512], FP32, tag="ps")
    for kc in range(2):
        nc.tensor.matmul(v_bank[:], fT[:, kc, :], wv_s[:, kc, :],
                         start=(kc == 0), stop=(kc == 1))
    s[(g, "qT_ps")] = qT_ps
    s[(g, "kT_ps")] = kT_ps
    s[(g, "v_ps")] = v_bank


def em_copies_tail(nc, sb, s, kT2, v2, g):
    qT_ps = s.pop((g, "qT_ps"))
    kT_ps = s.pop((g, "kT_ps"))
    v_ps = s.pop((g, "v_ps"))

    qT2 = sb.tile([128, 4, 2, PP], BF16, tag="qT2")
    for hp in range(2):
        nc.vector.tensor_copy(
            qT2[64 * hp:64 * hp + 64, :, :, :],
            qT_ps[64 * hp:64 * hp + 64, :, :, :])
    for hp in range(2):
        nc.scalar.copy(
            kT2[64 * hp:64 * hp + 64, :, :, 64 * hp:64 * hp + PP],
            kT_ps[64 * hp:64 * hp + 64, :, :, :].rearrange(
                "p nk w ts -> p w nk ts"))
    nc.scalar.copy(v2[:, 0, :, 0:D],
                   v_ps[0:PP, :].rearrange("p (h d) -> p h d", h=H))
    nc.vector.tensor_copy(v2[:, 1, :, 0:D],
                          v_ps[64:64 + PP, :].rearrange("p (h d) -> p h d", h=H))
    s[(g, "qT2")] = qT2
    s[(g, "kT2")] = kT2
    s[(g, "v2")] = v2


def em_dma_out(nc, s, out, c, g):
    img, wx, u = c
    fin = s.pop((g, "fin"))
    for w, eng in ((0, nc.sync), (1, nc.gpsimd)):
        c0 = P * (2 * u + w)
        eng.dma_start(out=out[img, P * wx:P * wx + P, c0:c0 + P, :],
                      in_=fin[64 * w:64 * w + PP, :])


_CACHED = {}


def _get_nc():
    if "nc" not in _CACHED:
        _CACHED["nc"] = build_bass()
    return _CACHED["nc"]


def kernel(fmap, Wq, Wkv, Wo, bo, _trace=False, _trace_kwargs=None):
    fmap = np.ascontiguousarray(fmap, dtype=np.float32)
    nc = _get_nc()
    in_maps = []
    for c in range(NCORES):
        in_maps.append({
            "fmap": fmap[IMGS_PER_CORE * c:IMGS_PER_CORE * (c + 1)],
            "Wq": np.ascontiguousarray(Wq, dtype=np.float32),
            "Wkv": np.ascontiguousarray(Wkv, dtype=np.float32),
            "Wo": np.ascontiguousarray(Wo, dtype=np.float32),
            "bo": np.ascontiguousarray(bo, dtype=np.float32),
        })
    res = run_bass_kernel_spmd(nc, in_maps, core_ids=list(range(NCORES)),
                               trace=_trace, **(_trace_kwargs or {}))
    outs = [r["out"] for r in res.results]
    full = np.concatenate(outs, axis=0)
    if _trace:
        return full, res
    return full


# revision 19
# speedup vs baseline: 4.8634x; 1.4649x over previous
"""Local window attention (7x7 windows, 8 heads, d=64) Trainium2 Bass kernel.

Full inputs in, full outputs out. Data-parallel over batch across 8 cores
(4 images per core). Shapes hardcoded per spec:
  fmap (32, 56, 56, 256) f32, Wq (256,512), Wkv (256,1024), Wo (512,256), bo (256,)

v3: software-pipelined stages (engines execute in-order per queue, and the
tile scheduler follows emission order, so cross-group overlap must be
explicit). Per group g = 2 adjacent-y windows, tokens 64-padded (p=64w+7r+t):

  P(g): 2 input DMAs (one per window, [49,256] <- [7,7,256])
  A(g): 2 PE transposes -> fT; q/k projections (16 mm); qT2/kT2 copies
        (kT2 block-diagonal over 2 heads, zero quadrants persist in
        manually-rotated buffers); v projection (2 mm); v2 copies (ones
        column persists at col 64 for the fused softmax denominator)
  B(g): ST 2-heads-per-matmul (8 mm); exp (2 ACT ops)
  C(g): av+denom (16 mm, N=65, lhsT=expS, rhs=[v|1]); tiny reciprocal of
        the denom column; normalize into out_tok via scalar_tensor_tensor
        with a stride-0 broadcast of 1/den; 4 PE transposes -> outT
  D(g): fin = outT.T @ Wo + bo (4 mm + stt); 2 output DMAs

Emission per iteration i: P(i+1), A(i), B(i-1), C(i-2), D(i-3) — 4 groups
in flight; PSUM allocs/iteration: A:4 B:1 C:3 D:1 = 9 on an 8-bank rotation.
All matmuls bf16 (psum fp32); casts ride the mandatory psum->SBUF copies.
"""

from contextlib import ExitStack

import numpy as np

import concourse.bacc as bacc
import concourse.bass as bass
import concourse.tile as tile
from concourse import mybir
from concourse import bass_isa
from concourse.masks import make_identity
from concourse.bass_utils import run_bass_kernel_spmd

P = 7
PP = 49          # tokens per window
H = 8            # heads
D = 64           # head dim
DIM = 256        # channels
INNER = 512      # h*d
SCALE = D ** -0.5
IMGS_PER_CORE = 4
NCORES = 8
X = 56
NW = X // P      # 8 windows per axis
FP32 = mybir.dt.float32
BF16 = mybir.dt.bfloat16
NROT = 4         # manual rotation depth for persistent tiles


def build_bass(n_imgs=IMGS_PER_CORE):
    nc = bacc.Bacc("TRN2", target_bir_lowering=False, debug=False)

    fm = nc.dram_tensor("fmap", [n_imgs, X, X, DIM], FP32, kind="ExternalInput").ap()
    wq = nc.dram_tensor("Wq", [DIM, INNER], FP32, kind="ExternalInput").ap()
    wkv = nc.dram_tensor("Wkv", [DIM, 2 * INNER], FP32, kind="ExternalInput").ap()
    wo = nc.dram_tensor("Wo", [INNER, DIM], FP32, kind="ExternalInput").ap()
    bo = nc.dram_tensor("bo", [DIM], FP32, kind="ExternalInput").ap()
    out = nc.dram_tensor("out", [n_imgs, X, X, DIM], FP32, kind="ExternalOutput").ap()

    with tile.TileContext(nc) as tc:
        with ExitStack() as ctx:
            build_kernel(ctx, tc, out, fm, wq, wkv, wo, bo, n_imgs)
    nc.compile()
    return nc


def build_kernel(ctx, tc, out, fm, wq, wkv, wo, bo, n_imgs=IMGS_PER_CORE):
    nc = tc.nc
    consts = ctx.enter_context(tc.tile_pool(name="consts", bufs=1))
    sb = ctx.enter_context(tc.tile_pool(name="sb", bufs=6))
    ps = ctx.enter_context(tc.tile_pool(name="ps", bufs=8, space="PSUM"))

    # ---- constants ----
    ident = consts.tile([128, 128], FP32)
    make_identity(nc, ident[:])

    ones = consts.tile([1, 128], FP32)
    nc.gpsimd.memset(ones[:], 1.0)
    identb = consts.tile([128, 128], BF16)
    nc.vector.tensor_copy(identb[:], ident[:])

    def stage_w(dram_ap, shape, name):
        st = sb.tile(shape, FP32, tag="stage")
        nc.sync.dma_start(out=st[:], in_=dram_ap)
        bt = consts.tile(shape, BF16, tag=name)
        nc.vector.tensor_copy(bt[:], st[:])
        return bt

    wq_s = stage_w(wq.rearrange("(kc ck) n -> ck kc n", ck=128), [128, 2, INNER],
                   "wq_s")
    wk_s = stage_w(wkv[:, 0:INNER].rearrange("(kc ck) n -> ck kc n", ck=128),
                   [128, 2, INNER], "wk_s")
    wv_s = stage_w(wkv[:, INNER:2 * INNER].rearrange("(kc ck) n -> ck kc n", ck=128),
                   [128, 2, INNER], "wv_s")
    wo_s = stage_w(wo.rearrange("(kc ck) m -> ck kc m", ck=128), [128, 4, DIM],
                   "wo_s")

    bo_f = consts.tile([1, DIM], FP32)
    nc.sync.dma_start(out=bo_f[:], in_=bo[None, :])
    bb_ps = ps.tile([128, 512], FP32, tag="ps")
    nc.tensor.matmul(bb_ps[:, 0:DIM], ones[0:1, :], bo_f[:], start=True, stop=True)
    bo_bc = consts.tile([128, DIM], FP32)
    nc.scalar.copy(bo_bc[:], bb_ps[:, 0:DIM])

    # persistent rotated buffers: kT2 (zero quadrants), v2 (ones column)
    kT2_bufs, v2_bufs = [], []
    for i in range(NROT):
        t = consts.tile([128, 2, 4, 128], BF16, tag=f"kT2_{i}")
        nc.gpsimd.memset(t[:], 0.0)
        kT2_bufs.append(t)
        v = consts.tile([PP, 2, H, D + 1], BF16, tag=f"v2_{i}")
        nc.gpsimd.memset(v[:, :, :, D:D + 1], 1.0)
        v2_bufs.append(v)

    # ---- software-pipelined main loop (v5: per-engine readiness order) ----
    # Group g's stages: P at iter g-1, A at g, B at g+1, C1 at g+2,
    # C2 at g+3, D at g+4. Within an iteration, ops are emitted so that
    # every engine's in-order queue meets its dependencies without stalling:
    # PE runs [transp(i), st(i-1), av(i-2), otT(i-3), fin(i-4), qk+v(i)],
    # ACT runs [fT(i), exp(i-1), outT(i-3), kT2(i)],
    # DVE runs [recd+norm(i-2), fin-stt(i-4), qT2(i), v2(i)].
    n_groups = n_imgs * NW * (NW // 2)

    def coords(g):
        img, rem = divmod(g, NW * (NW // 2))
        wx, u = divmod(rem, NW // 2)
        return img, wx, u

    s = {}  # cross-stage state, keyed (group, name)

    def live(g):
        return 0 <= g < n_groups

    for i in range(n_groups + 5):
        if i == 0:
            em_dma_in(nc, sb, s, fm, coords(0), 0)
        if live(i + 1):
            em_dma_in(nc, sb, s, fm, coords(i + 1), i + 1)
        if live(i - 1):
            em_copies_tail(nc, sb, s, kT2_bufs[(i - 1) % NROT],
                           v2_bufs[(i - 1) % NROT], i - 1)
        if live(i):
            em_pe_transp(nc, sb, ps, s, ident, i)
            em_act_ft(nc, sb, s, i)
        if live(i - 2):
            em_pe_av(nc, ps, s, i - 2)
            em_dve_norm(nc, sb, s, i - 2)
        if live(i - 3):
            em_pe_ot(nc, ps, s, identb, i - 3)
            em_act_outT(nc, sb, s, i - 3)
        if live(i - 4):
            em_pe_fin(nc, ps, s, wo_s, i - 4)
            em_dve_fin(nc, sb, s, bo_bc, i - 4)
        if live(i - 1):
            em_pe_st(nc, ps, s, i - 1)
            em_act_exp(nc, sb, s, i - 1)
        if live(i):
            em_pe_qkv(nc, ps, s, wq_s, wk_s, wv_s, i)
        if live(i - 4):
            em_dma_out(nc, s, out, coords(i - 4), i - 4)


def em_dma_in(nc, sb, s, fm, c, g):
    img, wx, u = c
    f_raw = sb.tile([128, DIM], FP32, tag="f_raw")
    for w, eng in ((0, nc.gpsimd), (1, nc.sync)):
        c0 = P * (2 * u + w)
        eng.dma_start(out=f_raw[64 * w:64 * w + PP, :],
                      in_=fm[img, P * wx:P * wx + P, c0:c0 + P, :])
    s[(g, "f_raw")] = f_raw


def em_pe_transp(nc, sb, ps, s, ident, g):
    f_raw = s.pop((g, "f_raw"))
    fT_bank = ps.tile([128, 512], FP32, tag="ps")
    fT_ps = fT_bank[:, 0:256].rearrange("p (kc t) -> p kc t", kc=2)
    for kc in range(2):
        nc.tensor.transpose(fT_ps[:, kc, 0:113],
                            f_raw[0:113, 128 * kc:128 * kc + 128],
                            ident[0:113, 0:113])
    s[(g, "fT_ps")] = fT_ps


def em_pe_st(nc, ps, s, g):
    qT2 = s.pop((g, "qT2"))
    kT2 = s.pop((g, "kT2"))
    st_bank = ps.tile([128, 512], FP32, tag="ps")
    st_ps = st_bank[:, 0:392].rearrange("p (w ch t) -> p w ch t", w=2, ch=4)
    for w in range(2):
        for ch in range(4):
            nc.tensor.matmul(st_ps[:, w, ch, :], kT2[:, w, ch, :],
                             qT2[:, ch, w, :], start=True, stop=True)
    s[(g, "st_ps")] = st_ps


def em_pe_av(nc, ps, s, g):
    expS = s.pop((g, "expS"))
    v2 = s.pop((g, "v2"))
    av_banks = []
    for w in range(2):
        avb = ps.tile([128, 512], FP32, tag="ps")
        av = avb[:, 0:260].rearrange("p (ch e) -> p ch e", ch=4)
        av_banks.append(av)
        for ch in range(4):
            for hp in range(2):
                h = 2 * ch + hp
                nc.tensor.matmul(
                    av[64 * hp:64 * hp + PP, ch, :],
                    expS[hp][:, w, ch, :],
                    v2[:, w, h, :],
                    tile_position=(0, 64 * hp), start=True, stop=True)
    s[(g, "av")] = av_banks


def em_pe_ot(nc, ps, s, identb, g):
    out_tok = s.pop((g, "out_tok"))
    ot_flat = out_tok[:].rearrange("p ch hp d -> p (ch hp d)")
    ot_bank = ps.tile([128, 1024], BF16, tag="ps")
    ot_ps = ot_bank[:, 0:512].rearrange("p (nk t) -> p nk t", nk=4)
    for nk in range(4):
        nc.tensor.transpose(ot_ps[:, nk, 0:113],
                            ot_flat[0:113, 128 * nk:128 * nk + 128],
                            identb[0:113, 0:113])
    s[(g, "ot_ps")] = ot_ps


def em_pe_fin(nc, ps, s, wo_s, g):
    outT = s.pop((g, "outT"))
    fin_bank = ps.tile([128, 512], FP32, tag="ps")
    fin_ps = fin_bank[:, 0:DIM]
    for nk in range(4):
        nc.tensor.matmul(fin_ps[:], outT[:, nk, :], wo_s[:, nk, :],
                         start=(nk == 0), stop=(nk == 3))
    s[(g, "fin_ps")] = fin_ps


def em_act_ft(nc, sb, s, g):
    fT_ps = s.pop((g, "fT_ps"))
    fT = sb.tile([128, 2, 128], BF16, tag="fT")
    nc.scalar.copy(fT[:], fT_ps[:])
    s[(g, "fT")] = fT


def em_act_exp(nc, sb, s, g):
    st_ps = s.pop((g, "st_ps"))
    expS = []
    for hp in range(2):
        e = sb.tile([PP, 2, 4, PP], BF16, tag=f"expS{hp}")
        nc.scalar.activation(e[:], st_ps[64 * hp:64 * hp + PP, :, :, :],
                             mybir.ActivationFunctionType.Exp, scale=SCALE)
        expS.append(e)
    s[(g, "expS")] = expS


def em_act_outT(nc, sb, s, g):
    ot_ps = s.pop((g, "ot_ps"))
    outT = sb.tile([128, 4, 128], BF16, tag="outT")
    nc.scalar.copy(outT[:], ot_ps[:])
    s[(g, "outT")] = outT


def em_dve_norm(nc, sb, s, g):
    av_banks = s.pop((g, "av"))
    out_tok = sb.tile([128, 4, 2, D], BF16, tag="out_tok")  # free = (ch, hp, d)
    for w in range(2):
        av = av_banks[w]
        recd = sb.tile([128, 4, 1], FP32, tag=f"recd{w}")
        nc.vector.reciprocal(recd[0:113, :, :], av[0:113, :, D:D + 1])
        for hp in range(2):
            nc.vector.scalar_tensor_tensor(
                out=out_tok[64 * w:64 * w + PP, :, hp, :],
                in0=av[64 * hp:64 * hp + PP, :, 0:D],
                scalar=1.0,
                in1=recd[64 * hp:64 * hp + PP, :, 0:1].broadcast_to((PP, 4, D)),
                op0=mybir.AluOpType.mult, op1=mybir.AluOpType.mult)
    s[(g, "out_tok")] = out_tok


def em_dve_fin(nc, sb, s, bo_bc, g):
    fin_ps = s.pop((g, "fin_ps"))
    fin = sb.tile([128, DIM], FP32, tag="fin")
    nc.vector.scalar_tensor_tensor(out=fin[:], in0=fin_ps[:], scalar=1.0,
                                   in1=bo_bc[:], op0=mybir.AluOpType.mult,
                                   op1=mybir.AluOpType.add)
    s[(g, "fin")] = fin


def em_pe_qkv(nc, ps, s, wq_s, wk_s, wv_s, g):
    fT = s.pop((g, "fT"))
    fT_c = fT[:].rearrange("p kc (w ts) -> p kc w ts", w=2)[:, :, :, 0:PP]
    q_bank = ps.tile([128, 512], FP32, tag="ps")
    qT_ps = q_bank[:, 0:392].rearrange("p (nk w t) -> p nk w t", nk=4, w=2)
    k_bank = ps.tile([128, 512], FP32, tag="ps")
    kT_ps = k_bank[:, 0:392].rearrange("p (nk w t) -> p nk w t", nk=4, w=2)
    for nk in range(4):
        for kc in range(2):
            nc.tensor.matmul(qT_ps[:, nk, :, :],
                             wq_s[:, kc, 128 * nk:128 * nk + 128],
                             fT_c[:, kc, :, :], start=(kc == 0), stop=(kc == 1))
            nc.tensor.matmul(kT_ps[:, nk, :, :],
                             wk_s[:, kc, 128 * nk:128 * nk + 128],
                             fT_c[:, kc, :, :], start=(kc == 0), stop=(kc == 1))
    v_bank = ps.tile([128, 512], FP32, tag="ps")
    for kc in range(2):
        nc.tensor.matmul(v_bank[:], fT[:, kc, :], wv_s[:, kc, :],
                         start=(kc == 0), stop=(kc == 1))
    s[(g, "qT_ps")] = qT_ps
    s[(g, "kT_ps")] = kT_ps
    s[(g, "v_ps")] = v_bank


def em_copies_tail(nc, sb, s, kT2, v2, g):
    qT_ps = s.pop((g, "qT_ps"))
    kT_ps = s.pop((g, "kT_ps"))
    v_ps = s.pop((g, "v_ps"))

    qT2 = sb.tile([128, 4, 2, PP], BF16, tag="qT2")
    for hp in range(2):
        nc.vector.tensor_copy(
            qT2[64 * hp:64 * hp + 64, :, :, :],
            qT_ps[64 * hp:64 * hp + 64, :, :, :])
    for hp in range(2):
        nc.scalar.copy(
            kT2[64 * hp:64 * hp + 64, :, :, 64 * hp:64 * hp + PP],
            kT_ps[64 * hp:64 * hp + 64, :, :, :].rearrange(
                "p nk w ts -> p w nk ts"))
    nc.scalar.copy(v2[:, 0, :, 0:D],
                   v_ps[0:PP, :].rearrange("p (h d) -> p h d", h=H))
    nc.vector.tensor_copy(v2[:, 1, :, 0:D],
                          v_ps[64:64 + PP, :].rearrange("p (h d) -> p h d", h=H))
    s[(g, "qT2")] = qT2
    s[(g, "kT2")] = kT2
    s[(g, "v2")] = v2


def em_dma_out(nc, s, out, c, g):
    img, wx, u = c
    fin = s.pop((g, "fin"))
    for w, eng in ((0, nc.sync), (1, nc.gpsimd)):
        c0 = P * (2 * u + w)
        eng.dma_start(out=out[img, P * wx:P * wx + P, c0:c0 + P, :],
                      in_=fin[64 * w:64 * w + PP, :])


_CACHED = {}


def _get_nc():
    if "nc" not in _CACHED:
        _CACHED["nc"] = build_bass()
    return _CACHED["nc"]


def kernel(fmap, Wq, Wkv, Wo, bo, _trace=False, _trace_kwargs=None):
    fmap = np.ascontiguousarray(fmap, dtype=np.float32)
    nc = _get_nc()
    in_maps = []
    for c in range(NCORES):
        in_maps.append({
            "fmap": fmap[IMGS_PER_CORE * c:IMGS_PER_CORE * (c + 1)],
            "Wq": np.ascontiguousarray(Wq, dtype=np.float32),
            "Wkv": np.ascontiguousarray(Wkv, dtype=np.float32),
            "Wo": np.ascontiguousarray(Wo, dtype=np.float32),
            "bo": np.ascontiguousarray(bo, dtype=np.float32),
        })
    res = run_bass_kernel_spmd(nc, in_maps, core_ids=list(range(NCORES)),
                               trace=_trace, **(_trace_kwargs or {}))
    outs = [r["out"] for r in res.results]
    full = np.concatenate(outs, axis=0)
    if _trace:
        return full, res
    return full
